# revision 33
# baseline (speedup 1.0000x reference)
"""Trainium2 Bass kernel for prefix-KV multi-head attention.

Reference computation (per batch):
    qkv = x @ w_qkv -> q,k,v heads; k/v get a 16-token prefix (pk, pv)
    attn = softmax(q @ k^T * D^-0.5); out = (attn @ v) @ w_proj + b_proj

Sharding: data-parallel over B across 8 NeuronCores (2 batches per core).
All matmul contractions land on the partition axis with no runtime data
reshuffling:

  x^T   [C, n]   via PE transposes of x
  q^T/k^T [f, n] = w_qkv-tile (stationary) x x^T (moving)
  v [n, f]       = x^T-tile (stationary) x w_v (moving)  -- token-major,
                   so attn@v stationaries need no transposes
  scores^T [m, n] per (head, m-tile): lhsT = k^T slice [64, 128]
  E^T = exp(scale * scores^T)  (ACT, reading PSUM directly)
  attn@v: lhsT = [v_h | 64 ones-columns] [m-tile, 128] -> psum rows 0:64 =
      unnormalized out^T, rows 64:128 = softmax denominator REPLICATED,
      accumulated over the 9 m-tiles (m-tile 0 = zero-padded prefix).
  out2^T = psum[0:64] * reciprocal_approx_fast(psum[64:128])
  final^T [c', n] = w_proj-tile (stationary) x out2^T (moving) + b (per-
      partition bias); host transposes the [C, N] result back to [N, C].

The attention inner loop is ACT(exp)-bound (~160us/batch of exp vs
~123us/batch of PE), so the two batches are software-pipelined: batch 1's
PE-only prep (x^T/k^T/q^T/v) is interleaved into batch 0's attention
window, and batch 0's projection into batch 1's attention window, keeping
the PE near-continuously busy (which also holds it at the 2.4 GHz
p-state; it drops toward 1.2 GHz after stalls).

This file is self-contained: it monkeypatches two workarounds for the
walrus build in this container (1-sync-wait-per-instruction cap).
"""

import json
import os
import sys

for _p in ("/opt/trn_rl_repo", os.path.expanduser("~/.axon_site/_ro/trn_rl_repo")):
    if os.path.isdir(_p) and _p not in sys.path:
        sys.path.insert(0, _p)

import numpy as np

import concourse.bass as bass
import concourse.tile as tile
from concourse import mybir
from concourse.bass_utils import run_bass_kernel_spmd
from concourse.vector_clock import ScopedClock
from concourse.masks import make_identity

F32 = mybir.dt.float32
F32R = mybir.dt.float32r
BF16 = mybir.dt.bfloat16
AF = mybir.ActivationFunctionType

# ---------------------------------------------------------------------------
# Workaround: this container's walrus supports at most ONE sync wait per
# instruction.  (a) split the TileContext-exit drain's waits onto single-wait
# NOPs; (b) at BIR-JSON serialization time, hoist extra waits from any
# instruction onto same-engine NOPs placed immediately before it.
# ---------------------------------------------------------------------------

def _patched_drain_and_barrier(self, tick_clock, wait_clock):
    drain_inst = self.nc.sync.drain()
    wait_clock.add_sem_waits(
        drain_inst.ins, ScopedClock({None: tick_clock.global_clock})
    )
    si = drain_inst.ins.sync_info
    waits = list(si.on_wait) if si is not None and si.on_wait else []
    if len(waits) > 1:
        si.on_wait = waits[:1]
        for w in waits[1:]:
            nop = self.nc.sync.nop(hint="drain_wait_split", nofuse=True)
            nsi = nop.ins.sync_info
            if nsi is None:
                nop.ins.sync_info = mybir.SyncInfo(on_wait=[w], on_update=[])
            else:
                nsi.on_wait = list(nsi.on_wait or []) + [w]
    self.nc.all_engine_barrier()
    assert self.sems is not None
    popped = self.nc._tile_sem_poison_stack.pop()
    assert popped is self._sem_poison
    self.nc.clear_and_free_semaphores(list(self.sems.allocated().values()))
    self.nc.all_engine_barrier()


tile.TileContext._drain_and_barrier = _patched_drain_and_barrier


def _split_multi_waits(bir):
    for fn in bir["functions"]:
        for bb in fn["blocks"]:
            new_insts = []
            for inst in bb["instructions"]:
                si = inst.get("sync_info")
                ow = (si or {}).get("on_wait") or []
                if len(ow) > 1:
                    for i, w in enumerate(ow[:-1]):
                        new_insts.append({
                            "debug": inst.get("debug", 0),
                            "engine": inst["engine"],
                            "ins": [], "outs": [],
                            "name": f"{inst['name']}.wsplit{i}",
                            "opcode": "NoOp",
                            "sync_info": {"on_wait": [w], "on_update": []},
                        })
                    si["on_wait"] = [ow[-1]]
                new_insts.append(inst)
            bb["instructions"] = new_insts
    return bir


_orig_to_json_bytes = bass.Bass.to_json_bytes


def _patched_to_json_bytes(self):
    d = json.loads(_orig_to_json_bytes(self))
    _split_multi_waits(d)
    return json.dumps(d).encode()


bass.Bass.to_json_bytes = _patched_to_json_bytes

# ---------------------------------------------------------------------------
# Activation-table pinning: the table-load insertion pass greedily picks the
# first act-func-set containing each function, so a kernel mixing Exp (bulk
# softmax) and Ln (reciprocal-via-exp(-ln)) thrashes between two table sets
# at 1283 ns per reload.  Restrict Exp to sets that also contain Ln, so one
# set serves every activation and exactly one table load is emitted.  (The
# set id indexes the real act_info.json, where that set does contain Exp, so
# walrus lowering is unaffected.)
# ---------------------------------------------------------------------------

import functools

import concourse.hw_specs as _hw_specs
import concourse.bacc as _bacc
import concourse.bass_interp as _bass_interp

_orig_get_act_tables = _hw_specs.get_activation_tables


@functools.cache
def _pinned_act_tables(module_arch):
    tabs = _orig_get_act_tables(module_arch)
    exp_fn = mybir.ActivationFunctionType.Exp
    ln_fn = mybir.ActivationFunctionType.Ln
    out = {}
    for name, fns in tabs.items():
        fns = set(fns)
        if exp_fn in fns and ln_fn not in fns:
            fns.discard(exp_fn)
        out[name] = fns
    return out


_hw_specs.get_activation_tables = _pinned_act_tables
_bacc.get_activation_tables = _pinned_act_tables
_bass_interp.get_activation_tables = _pinned_act_tables

# ---------------------------------------------------------------------------
# Problem constants (hardcoded per the task contract)
# ---------------------------------------------------------------------------

B, N, C, H, P = 16, 1024, 1024, 16, 16
D = C // H                      # 64
SCALE = float(D) ** -0.5        # 0.125
N_CORES = 8
B_PC = B // N_CORES             # 2 batches per core
NT = N // 128                   # 8 token tiles
CT = C // 128                   # 8 feature tiles
MT = NT + 1                     # 9 m-tiles: tile 0 = prefix (16 valid rows)
HPAIRS = H // 2                 # 8 head pairs (2 heads per 128-row f-tile)


def build_nc(repeat: int = 1) -> bass.Bass:
    nc = bass.Bass()

    x_d = nc.declare_dram_parameter("x", [B_PC, N, C], F32, isOutput=False)
    pk_d = nc.declare_dram_parameter("pk", [B_PC, P, C], F32, isOutput=False)
    pv_d = nc.declare_dram_parameter("pv", [B_PC, P, C], F32, isOutput=False)
    wqkv_d = nc.declare_dram_parameter("w_qkv", [C, 3 * C], F32, isOutput=False)
    wproj_d = nc.declare_dram_parameter("w_proj", [C, C], F32, isOutput=False)
    bias_d = nc.declare_dram_parameter("b_proj", [C], F32, isOutput=False)
    # output is stored TRANSPOSED per batch: [C, N]; host transposes back
    outT_d = nc.declare_dram_parameter("outT", [B_PC, C, N], F32, isOutput=True)
    # internal DRAM scratch: q^T spilled per batch during prep
    qsp_d = nc.dram_tensor("q_spill", [B_PC, CT, 128, N], BF16)

    from contextlib import ExitStack

    with tile.TileContext(nc) as tc:
        with ExitStack() as _stk:
            _pool = lambda *a, **kw: _stk.enter_context(tc.tile_pool(*a, **kw))
            cons = _pool(name="cons", bufs=1)
            wkv_pool = _pool(name="wkv", bufs=6)
            wv_pool = _pool(name="wv", bufs=2)
            wp_pool = _pool(name="wp", bufs=2)
            xload = _pool(name="xload", bufs=3)
            xT_pool = _pool(name="xT", bufs=2)
            kT_pool = _pool(name="kT", bufs=2)
            v_pool = _pool(name="vst", bufs=2)
            oT_pool = _pool(name="oT", bufs=2)
            qsb_pool = _pool(name="qsb", bufs=1)
            qp_pool = _pool(name="qp", bufs=2)
            e_pool = _pool(name="eT", bufs=3)
            vx_pool = _pool(name="vx", bufs=3)
            rb_pool = _pool(name="rb", bufs=1)
            stg_pool = _pool(name="stg", bufs=1)
            osb_pool = _pool(name="osb", bufs=2)
            ps_prep = _pool(name="psP", bufs=2, space="PSUM")
            ps_sc = _pool(name="psS", bufs=2, space="PSUM")
            ps_av_pool = _pool(name="psV", bufs=1, space="PSUM")

            ident = cons.tile([128, 128], F32, tag="ident")
            make_identity(nc, ident[:])
            # bias in per-partition layout: bias_col[p, cft] = b_proj[cft*128+p]
            bias_col = cons.tile([128, CT], F32, tag="bias")
            nc.sync.dma_start(
                out=bias_col[:],
                in_=bias_d[:].rearrange("(a b) -> b a", b=128),
            )

            def prep_chunks(rep, b):
                """Closures for batch-b prep: x^T, k^T, q^T(spill), v."""
                handles = {}
                handles["xT"] = xT_pool.tile(
                    [128, CT, N], BF16, tag="xT", name=f"xT_{rep}_{b}")
                handles["kT"] = kT_pool.tile(
                    [128, CT, MT * 128], BF16, tag="kT", name=f"kT_{rep}_{b}")
                handles["vst"] = v_pool.tile(
                    [128, NT, C], BF16, tag="vst", name=f"vst_{rep}_{b}")
                xT, kT, vst = handles["xT"], handles["kT"], handles["vst"]
                chunks = []
                xls = [None] * NT
                # streamed qkv weight tiles (bf16, gpsimd casting DMA),
                # prefetched ahead of their consumer chunks.  wkv tiles are
                # [p, ct, 128] stationaries (k then q f-tiles); wv tiles are
                # [p, ct, 512] moving halves for the token-major v matmuls.
                wks = [None] * CT
                wqs = [None] * CT
                wvs = [None] * 2

                def wk_load(ft):
                    wks[ft] = wkv_pool.tile([128, CT, 128], BF16, tag="wkv",
                                            name=f"wk_{rep}_{b}_{ft}")
                    nc.gpsimd.dma_start(
                        out=wks[ft][:],
                        in_=wqkv_d[:, C + ft * 128:C + (ft + 1) * 128
                                   ].rearrange("(co p) f -> p co f", p=128),
                    )

                def wq_load(ft):
                    wqs[ft] = wkv_pool.tile([128, CT, 128], BF16, tag="wkv",
                                            name=f"wq_{rep}_{b}_{ft}")
                    nc.gpsimd.dma_start(
                        out=wqs[ft][:],
                        in_=wqkv_d[:, ft * 128:(ft + 1) * 128
                                   ].rearrange("(co p) f -> p co f", p=128),
                    )

                def wv_load(fh):
                    wvs[fh] = wv_pool.tile([128, CT, 512], BF16, tag="wv",
                                           name=f"wv_{rep}_{b}_{fh}")
                    nc.gpsimd.dma_start(
                        out=wvs[fh][:],
                        in_=wqkv_d[:, 2 * C + fh * 512:2 * C + (fh + 1) * 512
                                   ].rearrange("(co p) f -> p co f", p=128),
                    )

                def lead():
                    for nt in range(2):
                        xls[nt] = xload.tile([128, C], F32, tag="xl", name=f"xl_{rep}_{b}_{nt}")
                        nc.sync.dma_start(
                            out=xls[nt][:],
                            in_=x_d[b, nt * 128:(nt + 1) * 128, :])
                    wk_load(0)
                    wq_load(0)
                    wv_load(0)
                chunks.append(lead)

                def xt_chunk(nt):
                    def go():
                        if nt + 2 < NT:
                            xls[nt + 2] = xload.tile([128, C], F32, tag="xl", name=f"xl_{rep}_{b}_{nt+2}")
                            nc.sync.dma_start(
                                out=xls[nt + 2][:],
                                in_=x_d[b, (nt + 2) * 128:(nt + 3) * 128, :])
                        for ch in range(2):
                            ps = ps_prep.tile([128, 512], F32, tag="pp")
                            for i in range(4):
                                ct = 4 * ch + i
                                nc.tensor.transpose(
                                    ps[:, i * 128:(i + 1) * 128],
                                    xls[nt][:, ct * 128:(ct + 1) * 128],
                                    ident[:],
                                )
                            nc.vector.tensor_copy(
                                xT[:, 4 * ch:4 * ch + 4,
                                   nt * 128:(nt + 1) * 128],
                                ps[:].rearrange("p (a c) -> p a c", c=128),
                            )
                    return go
                def prefix():
                    # zero prefix-tile pad columns 16..128 (scores -> 0,
                    # exp -> 1, but vx mt-0 pad rows are zero so no effect)
                    nc.vector.memset(kT[:, :, P:128], 0.0)
                    pkl = xload.tile([128, C], F32, tag="xl")
                    nc.sync.dma_start(out=pkl[0:P, :], in_=pk_d[b, :, :])
                    for ch in range(2):
                        ps = ps_prep.tile([128, 512], F32, tag="pp")
                        for i in range(4):
                            ct = 4 * ch + i
                            nc.tensor.transpose(
                                ps[:, i * 128:i * 128 + P],
                                pkl[0:P, ct * 128:(ct + 1) * 128],
                                ident[0:P, 0:P],
                            )
                        nc.vector.tensor_copy(
                            kT[:, 4 * ch:4 * ch + 4, 0:P],
                            ps[:].rearrange("p (a c) -> p a c", c=128)[
                                :, :, 0:P],
                        )
                def k_chunk(ft, nh):
                    def go():
                        if nh == 0 and ft + 1 < CT:
                            wk_load(ft + 1)
                        ps = ps_prep.tile([128, 512], F32, tag="pp")
                        for ct in range(CT):
                            nc.tensor.matmul(
                                ps[:],
                                wks[ft][:, ct, :],
                                xT[:, ct, nh * 512:(nh + 1) * 512],
                                start=(ct == 0), stop=(ct == CT - 1),
                            )
                        nc.vector.tensor_copy(
                            kT[:, ft, 128 + nh * 512:128 + (nh + 1) * 512],
                            ps[:],
                        )
                    return go
                # lead-in: x^T tiles with k(0)'s halves interleaved so
                # the PE has matmul work while the x DMAs land; attention
                # head h only reads kT f-tile h//2, q pair h//2 and v-half
                # h//8, so everything beyond k(0)/q(0)/v-half-0 is deferred
                # into the attention window itself
                for nt in range(4):
                    chunks.append(xt_chunk(nt))
                chunks.append(k_chunk(0, 0))
                for nt in range(4, NT):
                    chunks.append(xt_chunk(nt))
                chunks.append(prefix)
                chunks.append(k_chunk(0, 1))

                def q_chunk(ft):
                    def go():
                        if ft + 1 < CT:
                            wq_load(ft + 1)
                        q_sb = qsb_pool.tile([128, N], BF16, tag="qsb")
                        for nh in range(2):
                            ps = ps_prep.tile([128, 512], F32, tag="pp")
                            for ct in range(CT):
                                nc.tensor.matmul(
                                    ps[:],
                                    wqs[ft][:, ct, :],
                                    xT[:, ct, nh * 512:(nh + 1) * 512],
                                    start=(ct == 0), stop=(ct == CT - 1),
                                )
                            nc.vector.tensor_copy(
                                q_sb[:, nh * 512:(nh + 1) * 512], ps[:])
                        nc.sync.dma_start(out=qsp_d[b, ft], in_=q_sb[:])
                    return go
                chunks.append(q_chunk(0))

                def v_chunk(nt, fh):
                    def go():
                        # token-major v: psum[n, f] = xT-tile^T x w_v
                        ps = ps_prep.tile([128, 512], F32, tag="pp")
                        for ct in range(CT):
                            nc.tensor.matmul(
                                ps[:],
                                xT[:, ct, nt * 128:(nt + 1) * 128],
                                wvs[fh][:, ct, :],
                                start=(ct == 0), stop=(ct == CT - 1),
                            )
                        nc.vector.tensor_copy(
                            vst[:, nt, fh * 512:(fh + 1) * 512], ps[:])
                    return go
                for nt in range(NT):
                    chunks.append(v_chunk(nt, 0))

                # fillA: per-pair k/q triples, q first so the spill is
                # well ahead of the qp reload; deadline = head 2*ft
                fillA = []
                for ft in range(1, CT):
                    fillA.append(q_chunk(ft))
                    fillA.append(k_chunk(ft, 0))
                    fillA.append(k_chunk(ft, 1))
                # fillB: second v half, needed from head 8 on
                fillB = [lambda: wv_load(1)]
                for nt in range(NT):
                    fillB.append(v_chunk(nt, 1))

                return chunks, fillA, fillB, handles

            def attn_steps(rep, b, handles):
                """Generator: batch-b attention, one PE-step per yield."""
                kT, vst = handles["kT"], handles["vst"]
                oT = oT_pool.tile(
                    [128, CT, N], BF16, tag="oT", name=f"oT_{rep}_{b}")
                handles["oT"] = oT
                qps = [None] * HPAIRS

                def qp_load(pair):
                    qps[pair] = qp_pool.tile([128, N], BF16, tag="qp",
                                             name=f"qp_{rep}_{b}_{pair}")
                    nc.sync.dma_start(out=qps[pair][:], in_=qsp_d[b, pair])

                qp_load(0)
                for h in range(H):
                    hp, base = h // 2, 64 * (h % 2)
                    # prefetch the next pair's q at the ODD head: late
                    # enough that the deferred q-spill chunk feeding it has
                    # already been injected (same-queue RAW would deadlock),
                    # early enough to hide the DMA
                    if h % 2 == 1 and hp + 1 < HPAIRS:
                        qp_load(hp + 1)
                    qp = qps[hp]
                    # build vx for this head: [m, 0:64]=v_h, [m, 64:128]=ones
                    # (denominator trick); mt 0 rows P:128 are zero pads
                    vx = vx_pool.tile([128, MT, 128], BF16, tag="vx",
                                      name=f"vx_{rep}_{b}_{h}")
                    nc.vector.memset(vx[:, 0, :], 0.0)
                    nc.vector.memset(vx[:, 1:MT, 64:128], 1.0)
                    nc.vector.memset(vx[0:P, 0, 64:128], 1.0)
                    nc.gpsimd.dma_start(
                        out=vx[0:P, 0, 0:64],
                        in_=pv_d[b, :, h * D:(h + 1) * D],
                    )
                    nc.vector.tensor_copy(
                        vx[:, 1:MT, 0:64], vst[:, :, h * D:(h + 1) * D])

                    ps_av = ps_av_pool.tile([128, N], F32, tag="av",
                                            name=f"av_{rep}_{b}_{h}")
                    eTs = [None] * MT

                    def sc(mt):
                        ps = ps_sc.tile([128, N], F32, tag="sc")
                        for j in range(0, N, 512):
                            nc.tensor.matmul(
                                ps[:, j:j + 512],
                                kT[base:base + D, hp,
                                   mt * 128:(mt + 1) * 128],
                                qp[base:base + D, j:j + 512],
                                start=True, stop=True,
                            )
                        eTs[mt] = e_pool.tile([128, N], BF16, tag="eT", name=f"eT_{rep}_{b}_{h}_{mt}")
                        nc.scalar.activation(eTs[mt][:], ps[:], AF.Exp,
                                             scale=SCALE)

                    def av(mt):
                        for j in range(0, N, 512):
                            nc.tensor.matmul(
                                ps_av[:, j:j + 512],
                                vx[:, mt, :],
                                eTs[mt][:, j:j + 512],
                                start=(mt == 0), stop=(mt == MT - 1),
                            )
                        eTs[mt] = None

                    sc(0)
                    yield
                    for mt in range(1, MT):
                        sc(mt)
                        av(mt - 1)
                        yield
                    av(MT - 1)
                    # one copy frees the PSUM accumulator ASAP (next head's
                    # attn@v only waits on this); rows 64:128 hold the
                    # softmax denominator (replicated).  1/den is computed
                    # as exp(-ln(den)) on ACT -- both in one activation
                    # table set (see pinning patch above), no reloads --
                    # instead of the very slow DVE reciprocal.
                    stg = stg_pool.tile([128, N], F32, tag="stg")
                    nc.vector.tensor_copy(stg[:], ps_av[:])
                    rb = rb_pool.tile([64, N], F32, tag="rb")
                    nc.scalar.activation(rb[:], stg[64:128, :], AF.Ln)
                    nc.scalar.activation(rb[:], rb[:], AF.Exp, scale=-1.0)
                    nc.vector.tensor_mul(
                        oT[base:base + 64, hp, :], stg[0:64, :], rb[:])
                    yield

            def proj_chunks(rep, b, handles):
                oT = handles["oT"]
                chunks = []
                wps = [None] * CT

                def wp_load(cft):
                    wps[cft] = wp_pool.tile([128, CT, 128], BF16, tag="wp",
                                            name=f"wp_{rep}_{b}_{cft}")
                    nc.gpsimd.dma_start(
                        out=wps[cft][:],
                        in_=wproj_d[:, cft * 128:(cft + 1) * 128
                                    ].rearrange("(co p) f -> p co f", p=128),
                    )

                wp_load(0)

                def p_chunk(cft, nh):
                    def go():
                        # prefetch next tile's weights so the matmuls there
                        # never wait on their own DMA
                        if nh == 1 and cft + 1 < CT:
                            wp_load(cft + 1)
                        ps = ps_prep.tile([128, 512], F32, tag="pp")
                        for ct in range(CT):
                            nc.tensor.matmul(
                                ps[:],
                                wps[cft][:, ct, :],
                                oT[:, ct, nh * 512:(nh + 1) * 512],
                                start=(ct == 0), stop=(ct == CT - 1),
                            )
                        o_sb = osb_pool.tile([128, 512], F32, tag="osb")
                        nc.vector.tensor_scalar_add(
                            o_sb[:], ps[:], bias_col[:, cft:cft + 1])
                        nc.sync.dma_start(
                            out=outT_d[b, cft * 128:(cft + 1) * 128,
                                       nh * 512:(nh + 1) * 512],
                            in_=o_sb[:],
                        )
                    return go
                for cft in range(CT):
                    for nh in range(2):
                        chunks.append(p_chunk(cft, nh))
                return chunks

            def run_interleaved(steps, fills):
                """Emit attention steps; `fills` is a list of
                (chunk_list, cadence) pairs -- every `cadence` steps one
                chunk from that list is injected as PE filler for the
                ACT-bound stretches."""
                i = 0
                for _ in steps:
                    i += 1
                    for pair in fills:
                        fl, cad = pair
                        if fl and i % cad == 0:
                            fl.pop(0)()
                for fl, _ in fills:
                    for c in fl:
                        c()

            for rep in range(repeat):
                # window A: batch-0 minimal prep (x^T, k0, q0, v half 0) --
                # just enough for attention head 0 to start
                pre0, fa0, fb0, h0 = prep_chunks(rep, 0)
                for c in pre0:
                    c()
                # window B: batch-0 attention + (batch-0 late k/q/v,
                # batch-1 minimal prep) interleaved as PE filler
                pre1, fa1, fb1, h1 = prep_chunks(rep, 1)
                run_interleaved(attn_steps(rep, 0, h0),
                                [[fa0, 5], [fb0, 8], [pre1, 6]])
                # window C: batch-1 attention + (batch-1 late k/q/v,
                # batch-0 proj) interleaved
                run_interleaved(attn_steps(rep, 1, h1),
                                [[fa1, 5], [fb1, 8],
                                 [proj_chunks(rep, 0, h0), 9]])
                # window D: batch-1 proj, solo
                for c in proj_chunks(rep, 1, h1):
                    c()

    return nc


_NC_CACHE = {}


def _get_nc(repeat: int = 1) -> bass.Bass:
    key = f"nc{repeat}"
    if key not in _NC_CACHE:
        _NC_CACHE[key] = build_nc(repeat)
    return _NC_CACHE[key]


def _make_runner(nc):
    """Compile the SPMD kernel ONCE into a reusable callable.

    Mirrors bass2jax.run_bass_via_pjrt's multi-core branch, but without
    output-buffer donation so the compiled function + device-resident
    inputs can be invoked repeatedly (for wall-clock benchmarking and to
    avoid recompiles on every kernel() call).
    """
    import jax
    from jax.experimental.shard_map import shard_map
    from jax.sharding import Mesh, PartitionSpec
    from concourse import bass2jax
    from concourse.bass2jax import _bass_exec_p, partition_id_tensor

    bass2jax.install_neuronx_cc_hook()

    partition_name = (
        nc.partition_id_tensor.name if nc.partition_id_tensor else None
    )
    in_names, out_names, out_avals, zero_outs = [], [], [], []
    for alloc in nc.m.functions[0].allocations:
        if not isinstance(alloc, mybir.MemoryLocationSet):
            continue
        name = alloc.memorylocations[0].name
        if alloc.kind == "ExternalInput":
            if name != partition_name:
                in_names.append(name)
        elif alloc.kind == "ExternalOutput":
            shape = tuple(alloc.tensor_shape)
            dtype = mybir.dt.np(alloc.dtype)
            out_names.append(name)
            out_avals.append(jax.core.ShapedArray(shape, dtype))
            zero_outs.append(np.zeros(shape, dtype))
    n_params = len(in_names)
    all_in_names = list(in_names) + list(out_names)
    if partition_name is not None:
        all_in_names.append(partition_name)

    def _body(*args):
        operands = list(args)
        if partition_name is not None:
            operands.append(partition_id_tensor())
        outs = _bass_exec_p.bind(
            *operands,
            out_avals=tuple(out_avals),
            in_names=tuple(all_in_names),
            out_names=tuple(out_names),
            lowering_input_output_aliases=(),
            sim_require_finite=True,
            sim_require_nnan=True,
            nc=nc,
        )
        return tuple(outs)

    devices = jax.devices()[:N_CORES]
    mesh = Mesh(np.asarray(devices), ("core",))
    n_outs = len(out_avals)
    in_specs = (PartitionSpec("core"),) * (n_params + n_outs)
    out_specs = (PartitionSpec("core"),) * n_outs
    sharded = jax.jit(
        shard_map(_body, mesh=mesh, in_specs=in_specs,
                  out_specs=out_specs, check_rep=False),
        keep_unused=True,
    )

    concat_zeros = [
        np.zeros((N_CORES * z.shape[0], *z.shape[1:]), z.dtype)
        for z in zero_outs
    ]

    state = {"dev_zeros": None}

    def runner(in_maps):
        per_core = [
            [np.asarray(m[name]) for name in in_names] for m in in_maps
        ]
        concat_in = [
            np.concatenate([per_core[c][i] for c in range(N_CORES)], axis=0)
            for i in range(n_params)
        ]
        if state["dev_zeros"] is None:
            state["dev_zeros"] = [jax.device_put(z) for z in concat_zeros]
        out_arrs = sharded(*concat_in, *state["dev_zeros"])
        return [
            {
                name: np.asarray(out_arrs[i]).reshape(
                    N_CORES, *out_avals[i].shape
                )[c]
                for i, name in enumerate(out_names)
            }
            for c in range(N_CORES)
        ]

    def runner_dev(dev_args):
        """dev_args: device-resident concat inputs; returns device outputs."""
        return sharded(*dev_args, *state["dev_zeros"])

    def make_dev_args(in_maps):
        per_core = [
            [np.asarray(m[name]) for name in in_names] for m in in_maps
        ]
        concat_in = [
            np.concatenate([per_core[c][i] for c in range(N_CORES)], axis=0)
            for i in range(n_params)
        ]
        if state["dev_zeros"] is None:
            state["dev_zeros"] = [jax.device_put(z) for z in concat_zeros]
        return [jax.device_put(a) for a in concat_in]

    return runner, runner_dev, make_dev_args


def _get_runner(repeat: int = 1):
    key = f"runner{repeat}"
    if key not in _NC_CACHE:
        _NC_CACHE[key] = _make_runner(_get_nc(repeat))
    return _NC_CACHE[key]


def _make_in_maps(x, pk, pv, w_qkv, w_proj, b_proj):
    x = np.ascontiguousarray(np.asarray(x, dtype=np.float32))
    pk = np.ascontiguousarray(np.asarray(pk, dtype=np.float32))
    pv = np.ascontiguousarray(np.asarray(pv, dtype=np.float32))
    w_qkv = np.ascontiguousarray(np.asarray(w_qkv, dtype=np.float32))
    w_proj = np.ascontiguousarray(np.asarray(w_proj, dtype=np.float32))
    b_proj = np.ascontiguousarray(np.asarray(b_proj, dtype=np.float32))
    in_maps = []
    for c in range(N_CORES):
        sl = slice(c * B_PC, (c + 1) * B_PC)
        in_maps.append({
            "x": x[sl], "pk": pk[sl], "pv": pv[sl],
            "w_qkv": w_qkv, "w_proj": w_proj, "b_proj": b_proj,
        })
    return in_maps


def run(x, pk, pv, w_qkv, w_proj, b_proj, trace=False, **trace_kwargs):
    """Run the SPMD kernel; returns (output [B,N,C], per-core results).

    With trace=True, runs through run_bass_kernel_spmd so the NTFF
    profile hook captures HW exec time; returns (out, BassKernelResults).
    """
    in_maps = _make_in_maps(x, pk, pv, w_qkv, w_proj, b_proj)
    if trace:
        res = run_bass_kernel_spmd(
            _get_nc(), in_maps, core_ids=list(range(N_CORES)),
            trace=True, **trace_kwargs,
        )
        results = res.results
    else:
        runner, _, _ = _get_runner()
        results = runner(in_maps)
        res = results
    out = np.empty((B, N, C), dtype=np.float32)
    for c in range(N_CORES):
        outT = results[c]["outT"]              # [B_PC, C, N]
        out[c * B_PC:(c + 1) * B_PC] = outT.transpose(0, 2, 1)
    return out, res


def kernel(x, pk, pv, w_qkv, w_proj, b_proj) -> np.ndarray:
    out, _ = run(x, pk, pv, w_qkv, w_proj, b_proj)
    return out


def benchmark(x, pk, pv, w_qkv, w_proj, b_proj, iters=20, warmup=3, repeat=1):
    """Median wall-clock per executed call with device-resident inputs."""
    import time
    import jax
    _, runner_dev, make_dev_args = _get_runner(repeat)
    in_maps = _make_in_maps(x, pk, pv, w_qkv, w_proj, b_proj)
    dev_args = make_dev_args(in_maps)
    for _ in range(warmup):
        outs = runner_dev(dev_args)
        jax.block_until_ready(outs)
    ts = []
    for _ in range(iters):
        t0 = time.perf_counter()
        outs = runner_dev(dev_args)
        jax.block_until_ready(outs)
        ts.append(time.perf_counter() - t0)
    ts.sort()
    return {
        "median_s": ts[len(ts) // 2],
        "min_s": ts[0],
        "all_s": ts,
    }



# revision 34
# speedup vs baseline: 1.1600x; 1.1600x over previous
"""Trainium2 Bass kernel for prefix-KV multi-head attention.

Reference computation (per batch):
    qkv = x @ w_qkv -> q,k,v heads; k/v get a 16-token prefix (pk, pv)
    attn = softmax(q @ k^T * D^-0.5); out = (attn @ v) @ w_proj + b_proj

Sharding: data-parallel over B across 8 NeuronCores (2 batches per core).
All matmul contractions land on the partition axis with no runtime data
reshuffling:

  x^T   [C, n]   via PE transposes of x
  q^T/k^T [f, n] = w_qkv-tile (stationary) x x^T (moving)
  v [n, f]       = x^T-tile (stationary) x w_v (moving)  -- token-major,
                   so attn@v stationaries need no transposes
  scores^T [m, n] per (head, m-tile): lhsT = k^T slice [64, 128]
  E^T = exp(scale * scores^T)  (ACT, reading PSUM directly)
  attn@v: lhsT = [v_h | 64 ones-columns] [m-tile, 128] -> psum rows 0:64 =
      unnormalized out^T, rows 64:128 = softmax denominator REPLICATED,
      accumulated over the 9 m-tiles (m-tile 0 = zero-padded prefix).
  out2^T = psum[0:64] * reciprocal_approx_fast(psum[64:128])
  final^T [c', n] = w_proj-tile (stationary) x out2^T (moving) + b (per-
      partition bias); host transposes the [C, N] result back to [N, C].

The attention inner loop is ACT(exp)-bound (~160us/batch of exp vs
~123us/batch of PE), so the two batches are software-pipelined: batch 1's
PE-only prep (x^T/k^T/q^T/v) is interleaved into batch 0's attention
window, and batch 0's projection into batch 1's attention window, keeping
the PE near-continuously busy (which also holds it at the 2.4 GHz
p-state; it drops toward 1.2 GHz after stalls).

This file is self-contained: it monkeypatches two workarounds for the
walrus build in this container (1-sync-wait-per-instruction cap).
"""

import json
import os
import sys

for _p in ("/opt/trn_rl_repo", os.path.expanduser("~/.axon_site/_ro/trn_rl_repo")):
    if os.path.isdir(_p) and _p not in sys.path:
        sys.path.insert(0, _p)

import numpy as np

import concourse.bass as bass
import concourse.tile as tile
from concourse import mybir
from concourse.bass_utils import run_bass_kernel_spmd
from concourse.vector_clock import ScopedClock
from concourse.masks import make_identity

F32 = mybir.dt.float32
F32R = mybir.dt.float32r
BF16 = mybir.dt.bfloat16
AF = mybir.ActivationFunctionType

# ---------------------------------------------------------------------------
# Workaround: this container's walrus supports at most ONE sync wait per
# instruction.  (a) split the TileContext-exit drain's waits onto single-wait
# NOPs; (b) at BIR-JSON serialization time, hoist extra waits from any
# instruction onto same-engine NOPs placed immediately before it.
# ---------------------------------------------------------------------------

def _patched_drain_and_barrier(self, tick_clock, wait_clock):
    drain_inst = self.nc.sync.drain()
    wait_clock.add_sem_waits(
        drain_inst.ins, ScopedClock({None: tick_clock.global_clock})
    )
    si = drain_inst.ins.sync_info
    waits = list(si.on_wait) if si is not None and si.on_wait else []
    if len(waits) > 1:
        si.on_wait = waits[:1]
        for w in waits[1:]:
            nop = self.nc.sync.nop(hint="drain_wait_split", nofuse=True)
            nsi = nop.ins.sync_info
            if nsi is None:
                nop.ins.sync_info = mybir.SyncInfo(on_wait=[w], on_update=[])
            else:
                nsi.on_wait = list(nsi.on_wait or []) + [w]
    self.nc.all_engine_barrier()
    assert self.sems is not None
    popped = self.nc._tile_sem_poison_stack.pop()
    assert popped is self._sem_poison
    self.nc.clear_and_free_semaphores(list(self.sems.allocated().values()))
    self.nc.all_engine_barrier()


tile.TileContext._drain_and_barrier = _patched_drain_and_barrier


def _split_multi_waits(bir):
    for fn in bir["functions"]:
        for bb in fn["blocks"]:
            new_insts = []
            for inst in bb["instructions"]:
                si = inst.get("sync_info")
                ow = (si or {}).get("on_wait") or []
                if len(ow) > 1:
                    for i, w in enumerate(ow[:-1]):
                        new_insts.append({
                            "debug": inst.get("debug", 0),
                            "engine": inst["engine"],
                            "ins": [], "outs": [],
                            "name": f"{inst['name']}.wsplit{i}",
                            "opcode": "NoOp",
                            "sync_info": {"on_wait": [w], "on_update": []},
                        })
                    si["on_wait"] = [ow[-1]]
                new_insts.append(inst)
            bb["instructions"] = new_insts
    return bir


_orig_to_json_bytes = bass.Bass.to_json_bytes


def _patched_to_json_bytes(self):
    d = json.loads(_orig_to_json_bytes(self))
    _split_multi_waits(d)
    return json.dumps(d).encode()


bass.Bass.to_json_bytes = _patched_to_json_bytes

# ---------------------------------------------------------------------------
# Activation-table pinning: the table-load insertion pass greedily picks the
# first act-func-set containing each function, so a kernel mixing Exp (bulk
# softmax) and Ln (reciprocal-via-exp(-ln)) thrashes between two table sets
# at 1283 ns per reload.  Restrict Exp to sets that also contain Ln, so one
# set serves every activation and exactly one table load is emitted.  (The
# set id indexes the real act_info.json, where that set does contain Exp, so
# walrus lowering is unaffected.)
# ---------------------------------------------------------------------------

import functools

import concourse.hw_specs as _hw_specs
import concourse.bacc as _bacc
import concourse.bass_interp as _bass_interp

_orig_get_act_tables = _hw_specs.get_activation_tables


@functools.cache
def _pinned_act_tables(module_arch):
    tabs = _orig_get_act_tables(module_arch)
    exp_fn = mybir.ActivationFunctionType.Exp
    ln_fn = mybir.ActivationFunctionType.Ln
    out = {}
    for name, fns in tabs.items():
        fns = set(fns)
        if exp_fn in fns and ln_fn not in fns:
            fns.discard(exp_fn)
        out[name] = fns
    return out


_hw_specs.get_activation_tables = _pinned_act_tables
_bacc.get_activation_tables = _pinned_act_tables
_bass_interp.get_activation_tables = _pinned_act_tables

# ---------------------------------------------------------------------------
# Problem constants (hardcoded per the task contract)
# ---------------------------------------------------------------------------

B, N, C, H, P = 16, 1024, 1024, 16, 16
D = C // H                      # 64
SCALE = float(D) ** -0.5        # 0.125
N_CORES = 8
B_PC = B // N_CORES             # 2 batches per core
NT = N // 128                   # 8 token tiles
CT = C // 128                   # 8 feature tiles
MT = NT + 1                     # 9 m-tiles: tile 0 = prefix (16 valid rows)
HPAIRS = H // 2                 # 8 head pairs (2 heads per 128-row f-tile)


def build_nc(repeat: int = 1) -> bass.Bass:
    nc = bass.Bass()

    x_d = nc.declare_dram_parameter("x", [B_PC, N, C], F32, isOutput=False)
    pk_d = nc.declare_dram_parameter("pk", [B_PC, P, C], F32, isOutput=False)
    pv_d = nc.declare_dram_parameter("pv", [B_PC, P, C], F32, isOutput=False)
    wqkv_d = nc.declare_dram_parameter("w_qkv", [C, 3 * C], F32, isOutput=False)
    wproj_d = nc.declare_dram_parameter("w_proj", [C, C], F32, isOutput=False)
    bias_d = nc.declare_dram_parameter("b_proj", [C], F32, isOutput=False)
    # output is stored TRANSPOSED per batch: [C, N]; host transposes back
    outT_d = nc.declare_dram_parameter("outT", [B_PC, C, N], F32, isOutput=True)
    # internal DRAM scratch: q^T spilled per batch during prep
    qsp_d = nc.dram_tensor("q_spill", [B_PC, CT, 128, N], BF16)

    from contextlib import ExitStack

    with tile.TileContext(nc) as tc:
        with ExitStack() as _stk:
            _pool = lambda *a, **kw: _stk.enter_context(tc.tile_pool(*a, **kw))
            cons = _pool(name="cons", bufs=1)
            wkv_pool = _pool(name="wkv", bufs=10)
            wv_pool = _pool(name="wv", bufs=1)
            wp_pool = _pool(name="wp", bufs=2)
            xload = _pool(name="xload", bufs=3)
            xT_pool = _pool(name="xT", bufs=2)
            kT_pool = _pool(name="kT", bufs=2)
            v_pool = _pool(name="vst", bufs=2)
            oT_pool = _pool(name="oT", bufs=2)
            qsb_pool = _pool(name="qsb", bufs=1)
            qp_pool = _pool(name="qp", bufs=2)
            e_pool = _pool(name="eT", bufs=3)
            vx_pool = _pool(name="vx", bufs=3)
            rb_pool = _pool(name="rb", bufs=1)
            stg_pool = _pool(name="stg", bufs=1)
            osb_pool = _pool(name="osb", bufs=2)
            ps_prep = _pool(name="psP", bufs=2, space="PSUM")
            ps_sc = _pool(name="psS", bufs=2, space="PSUM")
            ps_av_pool = _pool(name="psV", bufs=1, space="PSUM")

            ident = cons.tile([128, 128], F32, tag="ident")
            make_identity(nc, ident[:])
            # bias in per-partition layout: bias_col[p, cft] = b_proj[cft*128+p]
            bias_col = cons.tile([128, CT], F32, tag="bias")
            nc.sync.dma_start(
                out=bias_col[:],
                in_=bias_d[:].rearrange("(a b) -> b a", b=128),
            )

            def prep_chunks(rep, b):
                """Closures for batch-b prep: x^T, k^T, q^T(spill), v."""
                handles = {}
                handles["xT"] = xT_pool.tile(
                    [128, CT, N], BF16, tag="xT", name=f"xT_{rep}_{b}")
                handles["kT"] = kT_pool.tile(
                    [128, CT, MT * 128], BF16, tag="kT", name=f"kT_{rep}_{b}")
                handles["vst"] = v_pool.tile(
                    [128, NT, C], BF16, tag="vst", name=f"vst_{rep}_{b}")
                xT, kT, vst = handles["xT"], handles["kT"], handles["vst"]
                chunks = []
                xls = [None] * NT
                # streamed qkv weight tiles (bf16, gpsimd casting DMA),
                # prefetched ahead of their consumer chunks.  wkv tiles are
                # [p, ct, 128] stationaries (k then q f-tiles); wv tiles are
                # [p, ct, 512] moving halves for the token-major v matmuls.
                wks = [None] * CT
                wqs = [None] * CT
                wvs = [None] * 2

                def wk_load(ft):
                    wks[ft] = wkv_pool.tile([128, CT, 128], BF16, tag="wkv",
                                            name=f"wk_{rep}_{b}_{ft}")
                    nc.gpsimd.dma_start(
                        out=wks[ft][:],
                        in_=wqkv_d[:, C + ft * 128:C + (ft + 1) * 128
                                   ].rearrange("(co p) f -> p co f", p=128),
                    )

                def wq_load(ft):
                    wqs[ft] = wkv_pool.tile([128, CT, 128], BF16, tag="wkv",
                                            name=f"wq_{rep}_{b}_{ft}")
                    nc.gpsimd.dma_start(
                        out=wqs[ft][:],
                        in_=wqkv_d[:, ft * 128:(ft + 1) * 128
                                   ].rearrange("(co p) f -> p co f", p=128),
                    )

                def wv_load(fh):
                    wvs[fh] = wv_pool.tile([128, CT, 512], BF16, tag="wv",
                                           name=f"wv_{rep}_{b}_{fh}")
                    nc.gpsimd.dma_start(
                        out=wvs[fh][:],
                        in_=wqkv_d[:, 2 * C + fh * 512:2 * C + (fh + 1) * 512
                                   ].rearrange("(co p) f -> p co f", p=128),
                    )

                def lead():
                    for nt in range(2):
                        xls[nt] = xload.tile([128, C], F32, tag="xl", name=f"xl_{rep}_{b}_{nt}")
                        nc.sync.dma_start(
                            out=xls[nt][:],
                            in_=x_d[b, nt * 128:(nt + 1) * 128, :])
                    wk_load(0)
                    wk_load(1)
                chunks.append(lead)

                def xt_chunk(nt):
                    def go():
                        if nt + 2 < NT:
                            xls[nt + 2] = xload.tile([128, C], F32, tag="xl", name=f"xl_{rep}_{b}_{nt+2}")
                            nc.sync.dma_start(
                                out=xls[nt + 2][:],
                                in_=x_d[b, (nt + 2) * 128:(nt + 3) * 128, :])
                        for ch in range(2):
                            ps = ps_prep.tile([128, 512], F32, tag="pp")
                            for i in range(4):
                                ct = 4 * ch + i
                                nc.tensor.transpose(
                                    ps[:, i * 128:(i + 1) * 128],
                                    xls[nt][:, ct * 128:(ct + 1) * 128],
                                    ident[:],
                                )
                            nc.vector.tensor_copy(
                                xT[:, 4 * ch:4 * ch + 4,
                                   nt * 128:(nt + 1) * 128],
                                ps[:].rearrange("p (a c) -> p a c", c=128),
                            )
                    return go
                def prefix():
                    # zero prefix-tile pad columns 16..128 (scores -> 0,
                    # exp -> 1, but vx mt-0 pad rows are zero so no effect)
                    nc.vector.memset(kT[:, :, P:128], 0.0)
                    pkl = xload.tile([128, C], F32, tag="xl")
                    nc.sync.dma_start(out=pkl[0:P, :], in_=pk_d[b, :, :])
                    for ch in range(2):
                        ps = ps_prep.tile([128, 512], F32, tag="pp")
                        for i in range(4):
                            ct = 4 * ch + i
                            nc.tensor.transpose(
                                ps[:, i * 128:i * 128 + P],
                                pkl[0:P, ct * 128:(ct + 1) * 128],
                                ident[0:P, 0:P],
                            )
                        nc.vector.tensor_copy(
                            kT[:, 4 * ch:4 * ch + 4, 0:P],
                            ps[:].rearrange("p (a c) -> p a c", c=128)[
                                :, :, 0:P],
                        )
                def k_chunk(ft, nh):
                    def go():
                        if nh == 0:
                            if ft + 2 < CT:
                                wk_load(ft + 2)
                            elif ft + 2 < CT + 2:
                                wq_load(ft + 2 - CT)
                        if ft == CT - 1 and nh == 0:
                            wv_load(0)
                        ps = ps_prep.tile([128, 512], F32, tag="pp")
                        for ct in range(CT):
                            nc.tensor.matmul(
                                ps[:],
                                wks[ft][:, ct, :],
                                xT[:, ct, nh * 512:(nh + 1) * 512],
                                start=(ct == 0), stop=(ct == CT - 1),
                            )
                        nc.vector.tensor_copy(
                            kT[:, ft, 128 + nh * 512:128 + (nh + 1) * 512],
                            ps[:],
                        )
                    return go
                # lead-in order: the first four x^T tiles, then k chunks
                # (n-half 0) interleaved with the remaining x^T tiles so the
                # PE has matmul work while x DMAs land
                for nt in range(4):
                    chunks.append(xt_chunk(nt))
                for i in range(4):
                    chunks.append(k_chunk(i, 0))
                    chunks.append(xt_chunk(4 + i))
                chunks.append(prefix)
                for ft in range(4, CT):
                    chunks.append(k_chunk(ft, 0))
                for ft in range(CT):
                    chunks.append(k_chunk(ft, 1))

                def q_chunk(ft):
                    def go():
                        if ft + 2 < CT:
                            wq_load(ft + 2)
                        q_sb = qsb_pool.tile([128, N], BF16, tag="qsb")
                        for nh in range(2):
                            ps = ps_prep.tile([128, 512], F32, tag="pp")
                            for ct in range(CT):
                                nc.tensor.matmul(
                                    ps[:],
                                    wqs[ft][:, ct, :],
                                    xT[:, ct, nh * 512:(nh + 1) * 512],
                                    start=(ct == 0), stop=(ct == CT - 1),
                                )
                            nc.vector.tensor_copy(
                                q_sb[:, nh * 512:(nh + 1) * 512], ps[:])
                        nc.sync.dma_start(out=qsp_d[b, ft], in_=q_sb[:])
                    return go
                chunks.append(q_chunk(0))

                def v_chunk(nt, fh):
                    def go():
                        # token-major v: psum[n, f] = xT-tile^T x w_v
                        ps = ps_prep.tile([128, 512], F32, tag="pp")
                        for ct in range(CT):
                            nc.tensor.matmul(
                                ps[:],
                                xT[:, ct, nt * 128:(nt + 1) * 128],
                                wvs[fh][:, ct, :],
                                start=(ct == 0), stop=(ct == CT - 1),
                            )
                        nc.vector.tensor_copy(
                            vst[:, nt, fh * 512:(fh + 1) * 512], ps[:])
                    return go
                for nt in range(NT):
                    chunks.append(v_chunk(nt, 0))

                # deferred chunks: only needed by later attention heads of
                # this batch, so they are injected into the attention window
                # itself (q(ft) feeds head pair ft; v fh=1 feeds heads 8-15)
                deferred = [q_chunk(1), q_chunk(2), lambda: wv_load(1),
                            q_chunk(3)]
                for nt in range(NT):
                    deferred.append(v_chunk(nt, 1))
                for ft in range(4, CT):
                    deferred.append(q_chunk(ft))

                return chunks, deferred, handles

            def attn_steps(rep, b, handles):
                """Generator: batch-b attention, one PE-step per yield."""
                kT, vst = handles["kT"], handles["vst"]
                oT = oT_pool.tile(
                    [128, CT, N], BF16, tag="oT", name=f"oT_{rep}_{b}")
                handles["oT"] = oT
                qps = [None] * HPAIRS

                def qp_load(pair):
                    qps[pair] = qp_pool.tile([128, N], BF16, tag="qp",
                                             name=f"qp_{rep}_{b}_{pair}")
                    nc.sync.dma_start(out=qps[pair][:], in_=qsp_d[b, pair])

                qp_load(0)
                for h in range(H):
                    hp, base = h // 2, 64 * (h % 2)
                    # prefetch the next pair's q at the ODD head: late
                    # enough that the deferred q-spill chunk feeding it has
                    # already been injected (same-queue RAW would deadlock),
                    # early enough to hide the DMA
                    if h % 2 == 1 and hp + 1 < HPAIRS:
                        qp_load(hp + 1)
                    qp = qps[hp]
                    # build vx for this head: [m, 0:64]=v_h, [m, 64:128]=ones
                    # (denominator trick); mt 0 rows P:128 are zero pads
                    vx = vx_pool.tile([128, MT, 128], BF16, tag="vx",
                                      name=f"vx_{rep}_{b}_{h}")
                    nc.vector.memset(vx[:, 0, :], 0.0)
                    nc.vector.memset(vx[:, 1:MT, 64:128], 1.0)
                    nc.vector.memset(vx[0:P, 0, 64:128], 1.0)
                    nc.gpsimd.dma_start(
                        out=vx[0:P, 0, 0:64],
                        in_=pv_d[b, :, h * D:(h + 1) * D],
                    )
                    nc.vector.tensor_copy(
                        vx[:, 1:MT, 0:64], vst[:, :, h * D:(h + 1) * D])

                    ps_av = ps_av_pool.tile([128, N], F32, tag="av",
                                            name=f"av_{rep}_{b}_{h}")
                    eTs = [None] * MT

                    def sc(mt):
                        ps = ps_sc.tile([128, N], F32, tag="sc")
                        for j in range(0, N, 512):
                            nc.tensor.matmul(
                                ps[:, j:j + 512],
                                kT[base:base + D, hp,
                                   mt * 128:(mt + 1) * 128],
                                qp[base:base + D, j:j + 512],
                                start=True, stop=True,
                            )
                        eTs[mt] = e_pool.tile([128, N], BF16, tag="eT", name=f"eT_{rep}_{b}_{h}_{mt}")
                        nc.scalar.activation(eTs[mt][:], ps[:], AF.Exp,
                                             scale=SCALE)

                    def av(mt):
                        for j in range(0, N, 512):
                            nc.tensor.matmul(
                                ps_av[:, j:j + 512],
                                vx[:, mt, :],
                                eTs[mt][:, j:j + 512],
                                start=(mt == 0), stop=(mt == MT - 1),
                            )
                        eTs[mt] = None

                    sc(0)
                    yield
                    for mt in range(1, MT):
                        sc(mt)
                        av(mt - 1)
                        yield
                    av(MT - 1)
                    # one copy frees the PSUM accumulator ASAP (next head's
                    # attn@v only waits on this); rows 64:128 hold the
                    # softmax denominator (replicated).  1/den is computed
                    # as exp(-ln(den)) on ACT -- both in one activation
                    # table set (see pinning patch above), no reloads --
                    # instead of the very slow DVE reciprocal.
                    stg = stg_pool.tile([128, N], F32, tag="stg")
                    nc.vector.tensor_copy(stg[:], ps_av[:])
                    rb = rb_pool.tile([64, N], F32, tag="rb")
                    nc.scalar.activation(rb[:], stg[64:128, :], AF.Ln)
                    nc.scalar.activation(rb[:], rb[:], AF.Exp, scale=-1.0)
                    nc.vector.tensor_mul(
                        oT[base:base + 64, hp, :], stg[0:64, :], rb[:])
                    yield

            def proj_chunks(rep, b, handles):
                oT = handles["oT"]
                chunks = []
                wps = [None] * CT

                def wp_load(cft):
                    wps[cft] = wp_pool.tile([128, CT, 128], BF16, tag="wp",
                                            name=f"wp_{rep}_{b}_{cft}")
                    nc.gpsimd.dma_start(
                        out=wps[cft][:],
                        in_=wproj_d[:, cft * 128:(cft + 1) * 128
                                    ].rearrange("(co p) f -> p co f", p=128),
                    )

                wp_load(0)

                def p_chunk(cft, nh):
                    def go():
                        # prefetch next tile's weights so the matmuls there
                        # never wait on their own DMA
                        if nh == 1 and cft + 1 < CT:
                            wp_load(cft + 1)
                        ps = ps_prep.tile([128, 512], F32, tag="pp")
                        for ct in range(CT):
                            nc.tensor.matmul(
                                ps[:],
                                wps[cft][:, ct, :],
                                oT[:, ct, nh * 512:(nh + 1) * 512],
                                start=(ct == 0), stop=(ct == CT - 1),
                            )
                        o_sb = osb_pool.tile([128, 512], F32, tag="osb")
                        nc.vector.tensor_scalar_add(
                            o_sb[:], ps[:], bias_col[:, cft:cft + 1])
                        nc.sync.dma_start(
                            out=outT_d[b, cft * 128:(cft + 1) * 128,
                                       nh * 512:(nh + 1) * 512],
                            in_=o_sb[:],
                        )
                    return go
                for cft in range(CT):
                    for nh in range(2):
                        chunks.append(p_chunk(cft, nh))
                return chunks

            def run_interleaved(steps, fills):
                """Emit attention steps; `fills` is a list of
                (chunk_list, cadence) pairs -- every `cadence` steps one
                chunk from that list is injected as PE filler for the
                ACT-bound stretches."""
                i = 0
                for _ in steps:
                    i += 1
                    for pair in fills:
                        fl, cad = pair
                        if fl and i % cad == 0:
                            fl.pop(0)()
                for fl, _ in fills:
                    for c in fl:
                        c()

            for rep in range(repeat):
                # window A: batch-0 early prep (x^T, k^T, q0, v half 0)
                pre0, def0, h0 = prep_chunks(rep, 0)
                for c in pre0:
                    c()
                # window B: batch-0 attention + (batch-0 late prep,
                # batch-1 early prep) interleaved
                pre1, def1, h1 = prep_chunks(rep, 1)
                run_interleaved(attn_steps(rep, 0, h0),
                                [[def0, 5], [pre1, 5]])
                # window C: batch-1 attention + (batch-1 late prep,
                # batch-0 proj halves) interleaved
                run_interleaved(attn_steps(rep, 1, h1),
                                [[def1, 5], [proj_chunks(rep, 0, h0), 5]])
                # window D: batch-1 proj, solo
                for c in proj_chunks(rep, 1, h1):
                    c()

    return nc


_NC_CACHE = {}


def _get_nc(repeat: int = 1) -> bass.Bass:
    key = f"nc{repeat}"
    if key not in _NC_CACHE:
        _NC_CACHE[key] = build_nc(repeat)
    return _NC_CACHE[key]


def _make_runner(nc):
    """Compile the SPMD kernel ONCE into a reusable callable.

    Mirrors bass2jax.run_bass_via_pjrt's multi-core branch, but without
    output-buffer donation so the compiled function + device-resident
    inputs can be invoked repeatedly (for wall-clock benchmarking and to
    avoid recompiles on every kernel() call).
    """
    import jax
    from jax.experimental.shard_map import shard_map
    from jax.sharding import Mesh, PartitionSpec
    from concourse import bass2jax
    from concourse.bass2jax import _bass_exec_p, partition_id_tensor

    bass2jax.install_neuronx_cc_hook()

    partition_name = (
        nc.partition_id_tensor.name if nc.partition_id_tensor else None
    )
    in_names, out_names, out_avals, zero_outs = [], [], [], []
    for alloc in nc.m.functions[0].allocations:
        if not isinstance(alloc, mybir.MemoryLocationSet):
            continue
        name = alloc.memorylocations[0].name
        if alloc.kind == "ExternalInput":
            if name != partition_name:
                in_names.append(name)
        elif alloc.kind == "ExternalOutput":
            shape = tuple(alloc.tensor_shape)
            dtype = mybir.dt.np(alloc.dtype)
            out_names.append(name)
            out_avals.append(jax.core.ShapedArray(shape, dtype))
            zero_outs.append(np.zeros(shape, dtype))
    n_params = len(in_names)
    all_in_names = list(in_names) + list(out_names)
    if partition_name is not None:
        all_in_names.append(partition_name)

    def _body(*args):
        operands = list(args)
        if partition_name is not None:
            operands.append(partition_id_tensor())
        outs = _bass_exec_p.bind(
            *operands,
            out_avals=tuple(out_avals),
            in_names=tuple(all_in_names),
            out_names=tuple(out_names),
            lowering_input_output_aliases=(),
            sim_require_finite=True,
            sim_require_nnan=True,
            nc=nc,
        )
        return tuple(outs)

    devices = jax.devices()[:N_CORES]
    mesh = Mesh(np.asarray(devices), ("core",))
    n_outs = len(out_avals)
    in_specs = (PartitionSpec("core"),) * (n_params + n_outs)
    out_specs = (PartitionSpec("core"),) * n_outs
    sharded = jax.jit(
        shard_map(_body, mesh=mesh, in_specs=in_specs,
                  out_specs=out_specs, check_rep=False),
        keep_unused=True,
    )

    concat_zeros = [
        np.zeros((N_CORES * z.shape[0], *z.shape[1:]), z.dtype)
        for z in zero_outs
    ]

    state = {"dev_zeros": None}

    def runner(in_maps):
        per_core = [
            [np.asarray(m[name]) for name in in_names] for m in in_maps
        ]
        concat_in = [
            np.concatenate([per_core[c][i] for c in range(N_CORES)], axis=0)
            for i in range(n_params)
        ]
        if state["dev_zeros"] is None:
            state["dev_zeros"] = [jax.device_put(z) for z in concat_zeros]
        out_arrs = sharded(*concat_in, *state["dev_zeros"])
        return [
            {
                name: np.asarray(out_arrs[i]).reshape(
                    N_CORES, *out_avals[i].shape
                )[c]
                for i, name in enumerate(out_names)
            }
            for c in range(N_CORES)
        ]

    def runner_dev(dev_args):
        """dev_args: device-resident concat inputs; returns device outputs."""
        return sharded(*dev_args, *state["dev_zeros"])

    def make_dev_args(in_maps):
        per_core = [
            [np.asarray(m[name]) for name in in_names] for m in in_maps
        ]
        concat_in = [
            np.concatenate([per_core[c][i] for c in range(N_CORES)], axis=0)
            for i in range(n_params)
        ]
        if state["dev_zeros"] is None:
            state["dev_zeros"] = [jax.device_put(z) for z in concat_zeros]
        return [jax.device_put(a) for a in concat_in]

    return runner, runner_dev, make_dev_args


def _get_runner(repeat: int = 1):
    key = f"runner{repeat}"
    if key not in _NC_CACHE:
        _NC_CACHE[key] = _make_runner(_get_nc(repeat))
    return _NC_CACHE[key]


def _make_in_maps(x, pk, pv, w_qkv, w_proj, b_proj):
    x = np.ascontiguousarray(np.asarray(x, dtype=np.float32))
    pk = np.ascontiguousarray(np.asarray(pk, dtype=np.float32))
    pv = np.ascontiguousarray(np.asarray(pv, dtype=np.float32))
    w_qkv = np.ascontiguousarray(np.asarray(w_qkv, dtype=np.float32))
    w_proj = np.ascontiguousarray(np.asarray(w_proj, dtype=np.float32))
    b_proj = np.ascontiguousarray(np.asarray(b_proj, dtype=np.float32))
    in_maps = []
    for c in range(N_CORES):
        sl = slice(c * B_PC, (c + 1) * B_PC)
        in_maps.append({
            "x": x[sl], "pk": pk[sl], "pv": pv[sl],
            "w_qkv": w_qkv, "w_proj": w_proj, "b_proj": b_proj,
        })
    return in_maps


def run(x, pk, pv, w_qkv, w_proj, b_proj, trace=False, **trace_kwargs):
    """Run the SPMD kernel; returns (output [B,N,C], per-core results).

    With trace=True, runs through run_bass_kernel_spmd so the NTFF
    profile hook captures HW exec time; returns (out, BassKernelResults).
    """
    in_maps = _make_in_maps(x, pk, pv, w_qkv, w_proj, b_proj)
    if trace:
        res = run_bass_kernel_spmd(
            _get_nc(), in_maps, core_ids=list(range(N_CORES)),
            trace=True, **trace_kwargs,
        )
        results = res.results
    else:
        runner, _, _ = _get_runner()
        results = runner(in_maps)
        res = results
    out = np.empty((B, N, C), dtype=np.float32)
    for c in range(N_CORES):
        outT = results[c]["outT"]              # [B_PC, C, N]
        out[c * B_PC:(c + 1) * B_PC] = outT.transpose(0, 2, 1)
    return out, res


def kernel(x, pk, pv, w_qkv, w_proj, b_proj) -> np.ndarray:
    out, _ = run(x, pk, pv, w_qkv, w_proj, b_proj)
    return out


def benchmark(x, pk, pv, w_qkv, w_proj, b_proj, iters=20, warmup=3, repeat=1):
    """Median wall-clock per executed call with device-resident inputs."""
    import time
    import jax
    _, runner_dev, make_dev_args = _get_runner(repeat)
    in_maps = _make_in_maps(x, pk, pv, w_qkv, w_proj, b_proj)
    dev_args = make_dev_args(in_maps)
    for _ in range(warmup):
        outs = runner_dev(dev_args)
        jax.block_until_ready(outs)
    ts = []
    for _ in range(iters):
        t0 = time.perf_counter()
        outs = runner_dev(dev_args)
        jax.block_until_ready(outs)
        ts.append(time.perf_counter() - t0)
    ts.sort()
    return {
        "median_s": ts[len(ts) // 2],
        "min_s": ts[0],
        "all_s": ts,
    }



# revision 36
# speedup vs baseline: 1.1926x; 1.0281x over previous
"""Trainium2 Bass kernel for prefix-KV multi-head attention.

Reference computation (per batch):
    qkv = x @ w_qkv -> q,k,v heads; k/v get a 16-token prefix (pk, pv)
    attn = softmax(q @ k^T * D^-0.5); out = (attn @ v) @ w_proj + b_proj

Sharding: data-parallel over B across 8 NeuronCores (2 batches per core).
All matmul contractions land on the partition axis with no runtime data
reshuffling:

  x^T   [C, n]   via PE transposes of x
  q^T/k^T [f, n] = w_qkv-tile (stationary) x x^T (moving)
  v [n, f]       = x^T-tile (stationary) x w_v (moving)  -- token-major,
                   so attn@v stationaries need no transposes
  scores^T [m, n] per (head, m-tile): lhsT = k^T slice [64, 128]
  E^T = exp(scale * scores^T)  (ACT, reading PSUM directly)
  attn@v: lhsT = [v_h | 64 ones-columns] [m-tile, 128] -> psum rows 0:64 =
      unnormalized out^T, rows 64:128 = softmax denominator REPLICATED,
      accumulated over the 9 m-tiles (m-tile 0 = zero-padded prefix).
  out2^T = psum[0:64] * reciprocal_approx_fast(psum[64:128])
  final^T [c', n] = w_proj-tile (stationary) x out2^T (moving) + b (per-
      partition bias); host transposes the [C, N] result back to [N, C].

The attention inner loop is ACT(exp)-bound (~160us/batch of exp vs
~123us/batch of PE), so the two batches are software-pipelined: batch 1's
PE-only prep (x^T/k^T/q^T/v) is interleaved into batch 0's attention
window, and batch 0's projection into batch 1's attention window, keeping
the PE near-continuously busy (which also holds it at the 2.4 GHz
p-state; it drops toward 1.2 GHz after stalls).

This file is self-contained: it monkeypatches two workarounds for the
walrus build in this container (1-sync-wait-per-instruction cap).
"""

import json
import os
import sys

for _p in ("/opt/trn_rl_repo", os.path.expanduser("~/.axon_site/_ro/trn_rl_repo")):
    if os.path.isdir(_p) and _p not in sys.path:
        sys.path.insert(0, _p)

import numpy as np

import concourse.bass as bass
import concourse.tile as tile
from concourse import mybir
from concourse.bass_utils import run_bass_kernel_spmd
from concourse.vector_clock import ScopedClock
from concourse.masks import make_identity

F32 = mybir.dt.float32
F32R = mybir.dt.float32r
BF16 = mybir.dt.bfloat16
AF = mybir.ActivationFunctionType

# ---------------------------------------------------------------------------
# Workaround: this container's walrus supports at most ONE sync wait per
# instruction.  (a) split the TileContext-exit drain's waits onto single-wait
# NOPs; (b) at BIR-JSON serialization time, hoist extra waits from any
# instruction onto same-engine NOPs placed immediately before it.
# ---------------------------------------------------------------------------

def _patched_drain_and_barrier(self, tick_clock, wait_clock):
    drain_inst = self.nc.sync.drain()
    wait_clock.add_sem_waits(
        drain_inst.ins, ScopedClock({None: tick_clock.global_clock})
    )
    si = drain_inst.ins.sync_info
    waits = list(si.on_wait) if si is not None and si.on_wait else []
    if len(waits) > 1:
        si.on_wait = waits[:1]
        for w in waits[1:]:
            nop = self.nc.sync.nop(hint="drain_wait_split", nofuse=True)
            nsi = nop.ins.sync_info
            if nsi is None:
                nop.ins.sync_info = mybir.SyncInfo(on_wait=[w], on_update=[])
            else:
                nsi.on_wait = list(nsi.on_wait or []) + [w]
    self.nc.all_engine_barrier()
    assert self.sems is not None
    popped = self.nc._tile_sem_poison_stack.pop()
    assert popped is self._sem_poison
    self.nc.clear_and_free_semaphores(list(self.sems.allocated().values()))
    self.nc.all_engine_barrier()


tile.TileContext._drain_and_barrier = _patched_drain_and_barrier


def _split_multi_waits(bir):
    for fn in bir["functions"]:
        for bb in fn["blocks"]:
            new_insts = []
            for inst in bb["instructions"]:
                si = inst.get("sync_info")
                ow = (si or {}).get("on_wait") or []
                if len(ow) > 1:
                    for i, w in enumerate(ow[:-1]):
                        new_insts.append({
                            "debug": inst.get("debug", 0),
                            "engine": inst["engine"],
                            "ins": [], "outs": [],
                            "name": f"{inst['name']}.wsplit{i}",
                            "opcode": "NoOp",
                            "sync_info": {"on_wait": [w], "on_update": []},
                        })
                    si["on_wait"] = [ow[-1]]
                new_insts.append(inst)
            bb["instructions"] = new_insts
    return bir


_orig_to_json_bytes = bass.Bass.to_json_bytes


def _patched_to_json_bytes(self):
    d = json.loads(_orig_to_json_bytes(self))
    _split_multi_waits(d)
    return json.dumps(d).encode()


bass.Bass.to_json_bytes = _patched_to_json_bytes

# ---------------------------------------------------------------------------
# Activation-table pinning: the table-load insertion pass greedily picks the
# first act-func-set containing each function, so a kernel mixing Exp (bulk
# softmax) and Ln (reciprocal-via-exp(-ln)) thrashes between two table sets
# at 1283 ns per reload.  Restrict Exp to sets that also contain Ln, so one
# set serves every activation and exactly one table load is emitted.  (The
# set id indexes the real act_info.json, where that set does contain Exp, so
# walrus lowering is unaffected.)
# ---------------------------------------------------------------------------

import functools

import concourse.hw_specs as _hw_specs
import concourse.bacc as _bacc
import concourse.bass_interp as _bass_interp

_orig_get_act_tables = _hw_specs.get_activation_tables


@functools.cache
def _pinned_act_tables(module_arch):
    tabs = _orig_get_act_tables(module_arch)
    exp_fn = mybir.ActivationFunctionType.Exp
    ln_fn = mybir.ActivationFunctionType.Ln
    out = {}
    for name, fns in tabs.items():
        fns = set(fns)
        if exp_fn in fns and ln_fn not in fns:
            fns.discard(exp_fn)
        out[name] = fns
    return out


_hw_specs.get_activation_tables = _pinned_act_tables
_bacc.get_activation_tables = _pinned_act_tables
_bass_interp.get_activation_tables = _pinned_act_tables

# ---------------------------------------------------------------------------
# Problem constants (hardcoded per the task contract)
# ---------------------------------------------------------------------------

B, N, C, H, P = 16, 1024, 1024, 16, 16
D = C // H                      # 64
SCALE = float(D) ** -0.5        # 0.125
N_CORES = 8
B_PC = B // N_CORES             # 2 batches per core
NT = N // 128                   # 8 token tiles
CT = C // 128                   # 8 feature tiles
MT = NT + 1                     # 9 m-tiles: tile 0 = prefix (16 valid rows)
HPAIRS = H // 2                 # 8 head pairs (2 heads per 128-row f-tile)


def build_nc(repeat: int = 1) -> bass.Bass:
    nc = bass.Bass()

    x_d = nc.declare_dram_parameter("x", [B_PC, N, C], F32, isOutput=False)
    pk_d = nc.declare_dram_parameter("pk", [B_PC, P, C], F32, isOutput=False)
    pv_d = nc.declare_dram_parameter("pv", [B_PC, P, C], F32, isOutput=False)
    wqkv_d = nc.declare_dram_parameter("w_qkv", [C, 3 * C], F32, isOutput=False)
    wproj_d = nc.declare_dram_parameter("w_proj", [C, C], F32, isOutput=False)
    bias_d = nc.declare_dram_parameter("b_proj", [C], F32, isOutput=False)
    # output is stored TRANSPOSED per batch: [C, N]; host transposes back
    outT_d = nc.declare_dram_parameter("outT", [B_PC, C, N], F32, isOutput=True)
    # internal DRAM scratch: q^T spilled per batch during prep
    qsp_d = nc.dram_tensor("q_spill", [B_PC, CT, 128, N], BF16)

    from contextlib import ExitStack

    with tile.TileContext(nc) as tc:
        with ExitStack() as _stk:
            _pool = lambda *a, **kw: _stk.enter_context(tc.tile_pool(*a, **kw))
            cons = _pool(name="cons", bufs=1)
            wkv_pool = _pool(name="wkv", bufs=10)
            wv_pool = _pool(name="wv", bufs=1)
            wp_pool = _pool(name="wp", bufs=2)
            xload = _pool(name="xload", bufs=3)
            xT_pool = _pool(name="xT", bufs=2)
            kT_pool = _pool(name="kT", bufs=2)
            v_pool = _pool(name="vst", bufs=2)
            oT_pool = _pool(name="oT", bufs=2)
            qsb_pool = _pool(name="qsb", bufs=1)
            qp_pool = _pool(name="qp", bufs=2)
            e_pool = _pool(name="eT", bufs=3)
            vx_pool = _pool(name="vx", bufs=3)
            rb_pool = _pool(name="rb", bufs=1)
            stg_pool = _pool(name="stg", bufs=1)
            osb_pool = _pool(name="osb", bufs=2)
            ps_prep = _pool(name="psP", bufs=2, space="PSUM")
            ps_sc = _pool(name="psS", bufs=2, space="PSUM")
            ps_av_pool = _pool(name="psV", bufs=1, space="PSUM")

            ident = cons.tile([128, 128], F32, tag="ident")
            make_identity(nc, ident[:])
            # bias in per-partition layout: bias_col[p, cft] = b_proj[cft*128+p]
            bias_col = cons.tile([128, CT], F32, tag="bias")
            nc.sync.dma_start(
                out=bias_col[:],
                in_=bias_d[:].rearrange("(a b) -> b a", b=128),
            )

            def prep_chunks(rep, b):
                """Closures for batch-b prep: x^T, k^T, q^T(spill), v."""
                handles = {}
                handles["xT"] = xT_pool.tile(
                    [128, CT, N], BF16, tag="xT", name=f"xT_{rep}_{b}")
                handles["kT"] = kT_pool.tile(
                    [128, CT, MT * 128], BF16, tag="kT", name=f"kT_{rep}_{b}")
                handles["vst"] = v_pool.tile(
                    [128, NT, C], BF16, tag="vst", name=f"vst_{rep}_{b}")
                xT, kT, vst = handles["xT"], handles["kT"], handles["vst"]
                chunks = []
                xls = [None] * NT
                # streamed qkv weight tiles (bf16, gpsimd casting DMA),
                # prefetched ahead of their consumer chunks.  wkv tiles are
                # [p, ct, 128] stationaries (k then q f-tiles); wv tiles are
                # [p, ct, 512] moving halves for the token-major v matmuls.
                wks = [None] * CT
                wqs = [None] * CT
                wvs = [None] * 2

                def wk_load(ft):
                    wks[ft] = wkv_pool.tile([128, CT, 128], BF16, tag="wkv",
                                            name=f"wk_{rep}_{b}_{ft}")
                    nc.gpsimd.dma_start(
                        out=wks[ft][:],
                        in_=wqkv_d[:, C + ft * 128:C + (ft + 1) * 128
                                   ].rearrange("(co p) f -> p co f", p=128),
                    )

                def wq_load(ft):
                    wqs[ft] = wkv_pool.tile([128, CT, 128], BF16, tag="wkv",
                                            name=f"wq_{rep}_{b}_{ft}")
                    nc.gpsimd.dma_start(
                        out=wqs[ft][:],
                        in_=wqkv_d[:, ft * 128:(ft + 1) * 128
                                   ].rearrange("(co p) f -> p co f", p=128),
                    )

                def wv_load(fh):
                    wvs[fh] = wv_pool.tile([128, CT, 512], BF16, tag="wv",
                                           name=f"wv_{rep}_{b}_{fh}")
                    nc.gpsimd.dma_start(
                        out=wvs[fh][:],
                        in_=wqkv_d[:, 2 * C + fh * 512:2 * C + (fh + 1) * 512
                                   ].rearrange("(co p) f -> p co f", p=128),
                    )

                def lead():
                    for nt in range(2):
                        xls[nt] = xload.tile([128, C], F32, tag="xl", name=f"xl_{rep}_{b}_{nt}")
                        nc.sync.dma_start(
                            out=xls[nt][:],
                            in_=x_d[b, nt * 128:(nt + 1) * 128, :])
                    wk_load(0)
                    wk_load(1)
                chunks.append(lead)

                def xt_chunk(nt):
                    def go():
                        if nt + 2 < NT:
                            xls[nt + 2] = xload.tile([128, C], F32, tag="xl", name=f"xl_{rep}_{b}_{nt+2}")
                            nc.sync.dma_start(
                                out=xls[nt + 2][:],
                                in_=x_d[b, (nt + 2) * 128:(nt + 3) * 128, :])
                        for ch in range(2):
                            ps = ps_prep.tile([128, 512], F32, tag="pp")
                            for i in range(4):
                                ct = 4 * ch + i
                                nc.tensor.transpose(
                                    ps[:, i * 128:(i + 1) * 128],
                                    xls[nt][:, ct * 128:(ct + 1) * 128],
                                    ident[:],
                                )
                            nc.vector.tensor_copy(
                                xT[:, 4 * ch:4 * ch + 4,
                                   nt * 128:(nt + 1) * 128],
                                ps[:].rearrange("p (a c) -> p a c", c=128),
                            )
                    return go
                def prefix():
                    wv_load(0)
                    # zero prefix-tile pad columns 16..128 (scores -> 0,
                    # exp -> 1, but vx mt-0 pad rows are zero so no effect)
                    nc.vector.memset(kT[:, :, P:128], 0.0)
                    pkl = xload.tile([128, C], F32, tag="xl")
                    nc.sync.dma_start(out=pkl[0:P, :], in_=pk_d[b, :, :])
                    for ch in range(2):
                        ps = ps_prep.tile([128, 512], F32, tag="pp")
                        for i in range(4):
                            ct = 4 * ch + i
                            nc.tensor.transpose(
                                ps[:, i * 128:i * 128 + P],
                                pkl[0:P, ct * 128:(ct + 1) * 128],
                                ident[0:P, 0:P],
                            )
                        nc.vector.tensor_copy(
                            kT[:, 4 * ch:4 * ch + 4, 0:P],
                            ps[:].rearrange("p (a c) -> p a c", c=128)[
                                :, :, 0:P],
                        )
                def k_chunk(ft, nh):
                    def go():
                        if nh == 0:
                            if ft < 2:
                                wk_load(ft + 2)        # wk2, wk3
                            elif ft < 4:
                                wq_load(ft - 2)        # wq0, wq1
                            elif ft == 4:
                                wk_load(7)
                        ps = ps_prep.tile([128, 512], F32, tag="pp")
                        for ct in range(CT):
                            nc.tensor.matmul(
                                ps[:],
                                wks[ft][:, ct, :],
                                xT[:, ct, nh * 512:(nh + 1) * 512],
                                start=(ct == 0), stop=(ct == CT - 1),
                            )
                        nc.vector.tensor_copy(
                            kT[:, ft, 128 + nh * 512:128 + (nh + 1) * 512],
                            ps[:],
                        )
                    return go
                # lead-in order: the first four x^T tiles, then k chunks
                # (n-half 0) interleaved with the remaining x^T tiles so the
                # PE has matmul work while x DMAs land.  k f-tiles 4-7 feed
                # attention heads 8+ only, so they are deferred into the
                # attention window.
                for nt in range(4):
                    chunks.append(xt_chunk(nt))
                for i in range(4):
                    chunks.append(k_chunk(i, 0))
                    chunks.append(xt_chunk(4 + i))
                chunks.append(prefix)
                for ft in range(4):
                    chunks.append(k_chunk(ft, 1))

                def q_chunk(ft):
                    def go():
                        if ft == 0:
                            wq_load(2)
                        elif ft < 4:
                            wk_load(ft + 3)            # wk4, wk5, wk6
                            wq_load(ft + 2)            # wq3, wq4, wq5
                        elif ft < 6:
                            wq_load(ft + 2)            # wq6, wq7
                        q_sb = qsb_pool.tile([128, N], BF16, tag="qsb")
                        for nh in range(2):
                            ps = ps_prep.tile([128, 512], F32, tag="pp")
                            for ct in range(CT):
                                nc.tensor.matmul(
                                    ps[:],
                                    wqs[ft][:, ct, :],
                                    xT[:, ct, nh * 512:(nh + 1) * 512],
                                    start=(ct == 0), stop=(ct == CT - 1),
                                )
                            nc.vector.tensor_copy(
                                q_sb[:, nh * 512:(nh + 1) * 512], ps[:])
                        nc.sync.dma_start(out=qsp_d[b, ft], in_=q_sb[:])
                    return go
                chunks.append(q_chunk(0))

                def v_chunk(nt, fh):
                    def go():
                        # token-major v: psum[n, f] = xT-tile^T x w_v
                        ps = ps_prep.tile([128, 512], F32, tag="pp")
                        for ct in range(CT):
                            nc.tensor.matmul(
                                ps[:],
                                xT[:, ct, nt * 128:(nt + 1) * 128],
                                wvs[fh][:, ct, :],
                                start=(ct == 0), stop=(ct == CT - 1),
                            )
                        nc.vector.tensor_copy(
                            vst[:, nt, fh * 512:(fh + 1) * 512], ps[:])
                    return go
                for nt in range(NT):
                    chunks.append(v_chunk(nt, 0))

                # deferred chunks: only needed by later attention heads of
                # this batch, so they are injected into the attention window
                # itself (q(ft)/k(ft) feed head pair ft; v fh=1 feeds heads
                # 8-15); ordered so every chunk lands before its deadline at
                # the standard cadence
                # q spills go EARLY (their qp reloads race through DRAM
                # scratch; keep tens of microseconds of margin), k/v chunks
                # follow, all meeting their head deadlines at cadence 4
                deferred = [q_chunk(1), q_chunk(2), lambda: wv_load(1),
                            q_chunk(3), v_chunk(0, 1), q_chunk(4),
                            v_chunk(1, 1), k_chunk(4, 0), v_chunk(2, 1),
                            k_chunk(4, 1), v_chunk(3, 1), q_chunk(5),
                            v_chunk(4, 1), v_chunk(5, 1), k_chunk(5, 0),
                            v_chunk(6, 1), k_chunk(5, 1), v_chunk(7, 1),
                            q_chunk(6), k_chunk(6, 0), k_chunk(6, 1),
                            q_chunk(7), k_chunk(7, 0), k_chunk(7, 1)]

                return chunks, deferred, handles

            def attn_steps(rep, b, handles):
                """Generator: batch-b attention, one PE-step per yield."""
                kT, vst = handles["kT"], handles["vst"]
                oT = oT_pool.tile(
                    [128, CT, N], BF16, tag="oT", name=f"oT_{rep}_{b}")
                handles["oT"] = oT
                qps = [None] * HPAIRS

                def qp_load(pair):
                    qps[pair] = qp_pool.tile([128, N], BF16, tag="qp",
                                             name=f"qp_{rep}_{b}_{pair}")
                    nc.sync.dma_start(out=qps[pair][:], in_=qsp_d[b, pair])

                qp_load(0)
                for h in range(H):
                    hp, base = h // 2, 64 * (h % 2)
                    # prefetch the next pair's q at the ODD head: late
                    # enough that the deferred q-spill chunk feeding it has
                    # already been injected (same-queue RAW would deadlock),
                    # early enough to hide the DMA
                    if h % 2 == 1 and hp + 1 < HPAIRS:
                        qp_load(hp + 1)
                    qp = qps[hp]
                    # build vx for this head: [m, 0:64]=v_h, [m, 64:128]=ones
                    # (denominator trick); mt 0 rows P:128 are zero pads
                    vx = vx_pool.tile([128, MT, 128], BF16, tag="vx",
                                      name=f"vx_{rep}_{b}_{h}")
                    nc.vector.memset(vx[:, 0, :], 0.0)
                    nc.vector.memset(vx[:, 1:MT, 64:128], 1.0)
                    nc.vector.memset(vx[0:P, 0, 64:128], 1.0)
                    nc.gpsimd.dma_start(
                        out=vx[0:P, 0, 0:64],
                        in_=pv_d[b, :, h * D:(h + 1) * D],
                    )
                    nc.vector.tensor_copy(
                        vx[:, 1:MT, 0:64], vst[:, :, h * D:(h + 1) * D])

                    ps_av = ps_av_pool.tile([128, N], F32, tag="av",
                                            name=f"av_{rep}_{b}_{h}")
                    eTs = [None] * MT

                    def sc(mt):
                        ps = ps_sc.tile([128, N], F32, tag="sc")
                        for j in range(0, N, 512):
                            nc.tensor.matmul(
                                ps[:, j:j + 512],
                                kT[base:base + D, hp,
                                   mt * 128:(mt + 1) * 128],
                                qp[base:base + D, j:j + 512],
                                start=True, stop=True,
                            )
                        eTs[mt] = e_pool.tile([128, N], BF16, tag="eT", name=f"eT_{rep}_{b}_{h}_{mt}")
                        nc.scalar.activation(eTs[mt][:], ps[:], AF.Exp,
                                             scale=SCALE)

                    def av(mt):
                        for j in range(0, N, 512):
                            nc.tensor.matmul(
                                ps_av[:, j:j + 512],
                                vx[:, mt, :],
                                eTs[mt][:, j:j + 512],
                                start=(mt == 0), stop=(mt == MT - 1),
                            )
                        eTs[mt] = None

                    sc(0)
                    yield
                    for mt in range(1, MT):
                        sc(mt)
                        av(mt - 1)
                        yield
                    av(MT - 1)
                    # one copy frees the PSUM accumulator ASAP (next head's
                    # attn@v only waits on this); rows 64:128 hold the
                    # softmax denominator (replicated).  1/den is computed
                    # as exp(-ln(den)) on ACT -- both in one activation
                    # table set (see pinning patch above), no reloads --
                    # instead of the very slow DVE reciprocal.
                    stg = stg_pool.tile([128, N], F32, tag="stg")
                    nc.vector.tensor_copy(stg[:], ps_av[:])
                    rb = rb_pool.tile([64, N], F32, tag="rb")
                    nc.scalar.activation(rb[:], stg[64:128, :], AF.Ln)
                    nc.scalar.activation(rb[:], rb[:], AF.Exp, scale=-1.0)
                    nc.vector.tensor_mul(
                        oT[base:base + 64, hp, :], stg[0:64, :], rb[:])
                    yield

            def proj_chunks(rep, b, handles):
                oT = handles["oT"]
                chunks = []
                wps = [None] * CT

                def wp_load(cft):
                    wps[cft] = wp_pool.tile([128, CT, 128], BF16, tag="wp",
                                            name=f"wp_{rep}_{b}_{cft}")
                    nc.gpsimd.dma_start(
                        out=wps[cft][:],
                        in_=wproj_d[:, cft * 128:(cft + 1) * 128
                                    ].rearrange("(co p) f -> p co f", p=128),
                    )

                wp_load(0)

                def p_chunk(cft, nh):
                    def go():
                        # prefetch next tile's weights so the matmuls there
                        # never wait on their own DMA
                        if nh == 1 and cft + 1 < CT:
                            wp_load(cft + 1)
                        ps = ps_prep.tile([128, 512], F32, tag="pp")
                        for ct in range(CT):
                            nc.tensor.matmul(
                                ps[:],
                                wps[cft][:, ct, :],
                                oT[:, ct, nh * 512:(nh + 1) * 512],
                                start=(ct == 0), stop=(ct == CT - 1),
                            )
                        o_sb = osb_pool.tile([128, 512], F32, tag="osb")
                        nc.vector.tensor_scalar_add(
                            o_sb[:], ps[:], bias_col[:, cft:cft + 1])
                        nc.sync.dma_start(
                            out=outT_d[b, cft * 128:(cft + 1) * 128,
                                       nh * 512:(nh + 1) * 512],
                            in_=o_sb[:],
                        )
                    return go
                for cft in range(CT):
                    for nh in range(2):
                        chunks.append(p_chunk(cft, nh))
                return chunks

            def run_interleaved(steps, fills):
                """Emit attention steps; `fills` is a list of
                (chunk_list, cadence) pairs -- every `cadence` steps one
                chunk from that list is injected as PE filler for the
                ACT-bound stretches."""
                i = 0
                for _ in steps:
                    i += 1
                    for pair in fills:
                        fl, cad = pair
                        if fl and i % cad == 0:
                            fl.pop(0)()
                for fl, _ in fills:
                    for c in fl:
                        c()

            for rep in range(repeat):
                # window A: batch-0 early prep (x^T, k^T, q0, v half 0)
                pre0, def0, h0 = prep_chunks(rep, 0)
                for c in pre0:
                    c()
                # window B: batch-0 attention + (batch-0 late prep,
                # batch-1 early prep) interleaved
                pre1, def1, h1 = prep_chunks(rep, 1)
                run_interleaved(attn_steps(rep, 0, h0),
                                [[def0, 4], [pre1, 5]])
                # window C: batch-1 attention + (batch-1 late prep,
                # batch-0 proj halves) interleaved
                run_interleaved(attn_steps(rep, 1, h1),
                                [[def1, 4], [proj_chunks(rep, 0, h0), 9]])
                # window D: batch-1 proj, solo
                for c in proj_chunks(rep, 1, h1):
                    c()

    return nc


_NC_CACHE = {}


def _get_nc(repeat: int = 1) -> bass.Bass:
    key = f"nc{repeat}"
    if key not in _NC_CACHE:
        _NC_CACHE[key] = build_nc(repeat)
    return _NC_CACHE[key]


def _make_runner(nc):
    """Compile the SPMD kernel ONCE into a reusable callable.

    Mirrors bass2jax.run_bass_via_pjrt's multi-core branch, but without
    output-buffer donation so the compiled function + device-resident
    inputs can be invoked repeatedly (for wall-clock benchmarking and to
    avoid recompiles on every kernel() call).
    """
    import jax
    from jax.experimental.shard_map import shard_map
    from jax.sharding import Mesh, PartitionSpec
    from concourse import bass2jax
    from concourse.bass2jax import _bass_exec_p, partition_id_tensor

    bass2jax.install_neuronx_cc_hook()

    partition_name = (
        nc.partition_id_tensor.name if nc.partition_id_tensor else None
    )
    in_names, out_names, out_avals, zero_outs = [], [], [], []
    for alloc in nc.m.functions[0].allocations:
        if not isinstance(alloc, mybir.MemoryLocationSet):
            continue
        name = alloc.memorylocations[0].name
        if alloc.kind == "ExternalInput":
            if name != partition_name:
                in_names.append(name)
        elif alloc.kind == "ExternalOutput":
            shape = tuple(alloc.tensor_shape)
            dtype = mybir.dt.np(alloc.dtype)
            out_names.append(name)
            out_avals.append(jax.core.ShapedArray(shape, dtype))
            zero_outs.append(np.zeros(shape, dtype))
    n_params = len(in_names)
    all_in_names = list(in_names) + list(out_names)
    if partition_name is not None:
        all_in_names.append(partition_name)

    def _body(*args):
        operands = list(args)
        if partition_name is not None:
            operands.append(partition_id_tensor())
        outs = _bass_exec_p.bind(
            *operands,
            out_avals=tuple(out_avals),
            in_names=tuple(all_in_names),
            out_names=tuple(out_names),
            lowering_input_output_aliases=(),
            sim_require_finite=True,
            sim_require_nnan=True,
            nc=nc,
        )
        return tuple(outs)

    devices = jax.devices()[:N_CORES]
    mesh = Mesh(np.asarray(devices), ("core",))
    n_outs = len(out_avals)
    in_specs = (PartitionSpec("core"),) * (n_params + n_outs)
    out_specs = (PartitionSpec("core"),) * n_outs
    sharded = jax.jit(
        shard_map(_body, mesh=mesh, in_specs=in_specs,
                  out_specs=out_specs, check_rep=False),
        keep_unused=True,
    )

    concat_zeros = [
        np.zeros((N_CORES * z.shape[0], *z.shape[1:]), z.dtype)
        for z in zero_outs
    ]

    state = {"dev_zeros": None}

    def runner(in_maps):
        per_core = [
            [np.asarray(m[name]) for name in in_names] for m in in_maps
        ]
        concat_in = [
            np.concatenate([per_core[c][i] for c in range(N_CORES)], axis=0)
            for i in range(n_params)
        ]
        if state["dev_zeros"] is None:
            state["dev_zeros"] = [jax.device_put(z) for z in concat_zeros]
        out_arrs = sharded(*concat_in, *state["dev_zeros"])
        return [
            {
                name: np.asarray(out_arrs[i]).reshape(
                    N_CORES, *out_avals[i].shape
                )[c]
                for i, name in enumerate(out_names)
            }
            for c in range(N_CORES)
        ]

    def runner_dev(dev_args):
        """dev_args: device-resident concat inputs; returns device outputs."""
        return sharded(*dev_args, *state["dev_zeros"])

    def make_dev_args(in_maps):
        per_core = [
            [np.asarray(m[name]) for name in in_names] for m in in_maps
        ]
        concat_in = [
            np.concatenate([per_core[c][i] for c in range(N_CORES)], axis=0)
            for i in range(n_params)
        ]
        if state["dev_zeros"] is None:
            state["dev_zeros"] = [jax.device_put(z) for z in concat_zeros]
        return [jax.device_put(a) for a in concat_in]

    return runner, runner_dev, make_dev_args


def _get_runner(repeat: int = 1):
    key = f"runner{repeat}"
    if key not in _NC_CACHE:
        _NC_CACHE[key] = _make_runner(_get_nc(repeat))
    return _NC_CACHE[key]


def _make_in_maps(x, pk, pv, w_qkv, w_proj, b_proj):
    x = np.ascontiguousarray(np.asarray(x, dtype=np.float32))
    pk = np.ascontiguousarray(np.asarray(pk, dtype=np.float32))
    pv = np.ascontiguousarray(np.asarray(pv, dtype=np.float32))
    w_qkv = np.ascontiguousarray(np.asarray(w_qkv, dtype=np.float32))
    w_proj = np.ascontiguousarray(np.asarray(w_proj, dtype=np.float32))
    b_proj = np.ascontiguousarray(np.asarray(b_proj, dtype=np.float32))
    in_maps = []
    for c in range(N_CORES):
        sl = slice(c * B_PC, (c + 1) * B_PC)
        in_maps.append({
            "x": x[sl], "pk": pk[sl], "pv": pv[sl],
            "w_qkv": w_qkv, "w_proj": w_proj, "b_proj": b_proj,
        })
    return in_maps


def run(x, pk, pv, w_qkv, w_proj, b_proj, trace=False, **trace_kwargs):
    """Run the SPMD kernel; returns (output [B,N,C], per-core results).

    With trace=True, runs through run_bass_kernel_spmd so the NTFF
    profile hook captures HW exec time; returns (out, BassKernelResults).
    """
    in_maps = _make_in_maps(x, pk, pv, w_qkv, w_proj, b_proj)
    if trace:
        res = run_bass_kernel_spmd(
            _get_nc(), in_maps, core_ids=list(range(N_CORES)),
            trace=True, **trace_kwargs,
        )
        results = res.results
    else:
        runner, _, _ = _get_runner()
        results = runner(in_maps)
        res = results
    out = np.empty((B, N, C), dtype=np.float32)
    for c in range(N_CORES):
        outT = results[c]["outT"]              # [B_PC, C, N]
        out[c * B_PC:(c + 1) * B_PC] = outT.transpose(0, 2, 1)
    return out, res


def kernel(x, pk, pv, w_qkv, w_proj, b_proj) -> np.ndarray:
    out, _ = run(x, pk, pv, w_qkv, w_proj, b_proj)
    return out


def benchmark(x, pk, pv, w_qkv, w_proj, b_proj, iters=20, warmup=3, repeat=1):
    """Median wall-clock per executed call with device-resident inputs."""
    import time
    import jax
    _, runner_dev, make_dev_args = _get_runner(repeat)
    in_maps = _make_in_maps(x, pk, pv, w_qkv, w_proj, b_proj)
    dev_args = make_dev_args(in_maps)
    for _ in range(warmup):
        outs = runner_dev(dev_args)
        jax.block_until_ready(outs)
    ts = []
    for _ in range(iters):
        t0 = time.perf_counter()
        outs = runner_dev(dev_args)
        jax.block_until_ready(outs)
        ts.append(time.perf_counter() - t0)
    ts.sort()
    return {
        "median_s": ts[len(ts) // 2],
        "min_s": ts[0],
        "all_s": ts,
    }



# revision 37
# speedup vs baseline: 1.2328x; 1.0337x over previous
"""Trainium2 Bass kernel for prefix-KV multi-head attention.

Reference computation (per batch):
    qkv = x @ w_qkv -> q,k,v heads; k/v get a 16-token prefix (pk, pv)
    attn = softmax(q @ k^T * D^-0.5); out = (attn @ v) @ w_proj + b_proj

Sharding: data-parallel over B across 8 NeuronCores (2 batches per core).
All matmul contractions land on the partition axis with no runtime data
reshuffling:

  x^T   [C, n]   via PE transposes of x
  q^T/k^T [f, n] = w_qkv-tile (stationary) x x^T (moving)
  v [n, f]       = x^T-tile (stationary) x w_v (moving)  -- token-major,
                   so attn@v stationaries need no transposes
  scores^T [m, n] per (head, m-tile): lhsT = k^T slice [64, 128]
  E^T = exp(scale * scores^T)  (ACT, reading PSUM directly)
  attn@v: lhsT = [v_h | 64 ones-columns] [m-tile, 128] -> psum rows 0:64 =
      unnormalized out^T, rows 64:128 = softmax denominator REPLICATED,
      accumulated over the 9 m-tiles (m-tile 0 = zero-padded prefix).
  out2^T = psum[0:64] * reciprocal_approx_fast(psum[64:128])
  final^T [c', n] = w_proj-tile (stationary) x out2^T (moving) + b (per-
      partition bias); host transposes the [C, N] result back to [N, C].

The attention inner loop is ACT(exp)-bound (~160us/batch of exp vs
~123us/batch of PE), so the two batches are software-pipelined: batch 1's
PE-only prep (x^T/k^T/q^T/v) is interleaved into batch 0's attention
window, and batch 0's projection into batch 1's attention window, keeping
the PE near-continuously busy (which also holds it at the 2.4 GHz
p-state; it drops toward 1.2 GHz after stalls).

This file is self-contained: it monkeypatches two workarounds for the
walrus build in this container (1-sync-wait-per-instruction cap).
"""

import json
import os
import sys

for _p in ("/opt/trn_rl_repo", os.path.expanduser("~/.axon_site/_ro/trn_rl_repo")):
    if os.path.isdir(_p) and _p not in sys.path:
        sys.path.insert(0, _p)

import numpy as np

import concourse.bass as bass
import concourse.tile as tile
from concourse import mybir
from concourse.bass_utils import run_bass_kernel_spmd
from concourse.vector_clock import ScopedClock
from concourse.masks import make_identity

F32 = mybir.dt.float32
F32R = mybir.dt.float32r
BF16 = mybir.dt.bfloat16
AF = mybir.ActivationFunctionType

# ---------------------------------------------------------------------------
# Workaround: this container's walrus supports at most ONE sync wait per
# instruction.  (a) split the TileContext-exit drain's waits onto single-wait
# NOPs; (b) at BIR-JSON serialization time, hoist extra waits from any
# instruction onto same-engine NOPs placed immediately before it.
# ---------------------------------------------------------------------------

def _patched_drain_and_barrier(self, tick_clock, wait_clock):
    drain_inst = self.nc.sync.drain()
    wait_clock.add_sem_waits(
        drain_inst.ins, ScopedClock({None: tick_clock.global_clock})
    )
    si = drain_inst.ins.sync_info
    waits = list(si.on_wait) if si is not None and si.on_wait else []
    if len(waits) > 1:
        si.on_wait = waits[:1]
        for w in waits[1:]:
            nop = self.nc.sync.nop(hint="drain_wait_split", nofuse=True)
            nsi = nop.ins.sync_info
            if nsi is None:
                nop.ins.sync_info = mybir.SyncInfo(on_wait=[w], on_update=[])
            else:
                nsi.on_wait = list(nsi.on_wait or []) + [w]
    self.nc.all_engine_barrier()
    assert self.sems is not None
    popped = self.nc._tile_sem_poison_stack.pop()
    assert popped is self._sem_poison
    self.nc.clear_and_free_semaphores(list(self.sems.allocated().values()))
    self.nc.all_engine_barrier()


tile.TileContext._drain_and_barrier = _patched_drain_and_barrier


def _split_multi_waits(bir):
    for fn in bir["functions"]:
        for bb in fn["blocks"]:
            new_insts = []
            for inst in bb["instructions"]:
                si = inst.get("sync_info")
                ow = (si or {}).get("on_wait") or []
                if len(ow) > 1:
                    for i, w in enumerate(ow[:-1]):
                        new_insts.append({
                            "debug": inst.get("debug", 0),
                            "engine": inst["engine"],
                            "ins": [], "outs": [],
                            "name": f"{inst['name']}.wsplit{i}",
                            "opcode": "NoOp",
                            "sync_info": {"on_wait": [w], "on_update": []},
                        })
                    si["on_wait"] = [ow[-1]]
                new_insts.append(inst)
            bb["instructions"] = new_insts
    return bir


_orig_to_json_bytes = bass.Bass.to_json_bytes


def _patched_to_json_bytes(self):
    d = json.loads(_orig_to_json_bytes(self))
    _split_multi_waits(d)
    return json.dumps(d).encode()


bass.Bass.to_json_bytes = _patched_to_json_bytes

# ---------------------------------------------------------------------------
# Activation-table pinning: the table-load insertion pass greedily picks the
# first act-func-set containing each function, so a kernel mixing Exp (bulk
# softmax) and Ln (reciprocal-via-exp(-ln)) thrashes between two table sets
# at 1283 ns per reload.  Restrict Exp to sets that also contain Ln, so one
# set serves every activation and exactly one table load is emitted.  (The
# set id indexes the real act_info.json, where that set does contain Exp, so
# walrus lowering is unaffected.)
# ---------------------------------------------------------------------------

import functools

import concourse.hw_specs as _hw_specs
import concourse.bacc as _bacc
import concourse.bass_interp as _bass_interp

_orig_get_act_tables = _hw_specs.get_activation_tables


@functools.cache
def _pinned_act_tables(module_arch):
    tabs = _orig_get_act_tables(module_arch)
    exp_fn = mybir.ActivationFunctionType.Exp
    ln_fn = mybir.ActivationFunctionType.Ln
    out = {}
    for name, fns in tabs.items():
        fns = set(fns)
        if exp_fn in fns and ln_fn not in fns:
            fns.discard(exp_fn)
        out[name] = fns
    return out


_hw_specs.get_activation_tables = _pinned_act_tables
_bacc.get_activation_tables = _pinned_act_tables
_bass_interp.get_activation_tables = _pinned_act_tables

# ---------------------------------------------------------------------------
# Problem constants (hardcoded per the task contract)
# ---------------------------------------------------------------------------

B, N, C, H, P = 16, 1024, 1024, 16, 16
D = C // H                      # 64
SCALE = float(D) ** -0.5        # 0.125
N_CORES = 8
B_PC = B // N_CORES             # 2 batches per core
NT = N // 128                   # 8 token tiles
CT = C // 128                   # 8 feature tiles
MT = NT + 1                     # 9 m-tiles: tile 0 = prefix (16 valid rows)
HPAIRS = H // 2                 # 8 head pairs (2 heads per 128-row f-tile)


def build_nc(repeat: int = 1) -> bass.Bass:
    nc = bass.Bass()

    x_d = nc.declare_dram_parameter("x", [B_PC, N, C], F32, isOutput=False)
    pk_d = nc.declare_dram_parameter("pk", [B_PC, P, C], F32, isOutput=False)
    pv_d = nc.declare_dram_parameter("pv", [B_PC, P, C], F32, isOutput=False)
    wqkv_d = nc.declare_dram_parameter("w_qkv", [C, 3 * C], F32, isOutput=False)
    wproj_d = nc.declare_dram_parameter("w_proj", [C, C], F32, isOutput=False)
    bias_d = nc.declare_dram_parameter("b_proj", [C], F32, isOutput=False)
    # output is stored TRANSPOSED per batch: [C, N]; host transposes back
    outT_d = nc.declare_dram_parameter("outT", [B_PC, C, N], F32, isOutput=True)
    # internal DRAM scratch: q^T spilled per batch during prep
    qsp_d = nc.dram_tensor("q_spill", [B_PC, CT, 128, N], BF16)

    from contextlib import ExitStack

    with tile.TileContext(nc) as tc:
        with ExitStack() as _stk:
            _pool = lambda *a, **kw: _stk.enter_context(tc.tile_pool(*a, **kw))
            cons = _pool(name="cons", bufs=1)
            wkv_pool = _pool(name="wkv", bufs=10)
            wv_pool = _pool(name="wv", bufs=1)
            wp_pool = _pool(name="wp", bufs=2)
            xload = _pool(name="xload", bufs=3)
            xT_pool = _pool(name="xT", bufs=2)
            kT_pool = _pool(name="kT", bufs=2)
            v_pool = _pool(name="vst", bufs=2)
            oT_pool = _pool(name="oT", bufs=2)
            qsb_pool = _pool(name="qsb", bufs=1)
            qp_pool = _pool(name="qp", bufs=2)
            e_pool = _pool(name="eT", bufs=3)
            vx_pool = _pool(name="vx", bufs=3)
            rb_pool = _pool(name="rb", bufs=1)
            stg_pool = _pool(name="stg", bufs=1)
            osb_pool = _pool(name="osb", bufs=2)
            ps_prep = _pool(name="psP", bufs=2, space="PSUM")
            ps_sc = _pool(name="psS", bufs=2, space="PSUM")
            ps_av_pool = _pool(name="psV", bufs=1, space="PSUM")

            ident = cons.tile([128, 128], F32, tag="ident")
            make_identity(nc, ident[:])
            # bias in per-partition layout: bias_col[p, cft] = b_proj[cft*128+p]
            bias_col = cons.tile([128, CT], F32, tag="bias")
            nc.sync.dma_start(
                out=bias_col[:],
                in_=bias_d[:].rearrange("(a b) -> b a", b=128),
            )

            def prep_chunks(rep, b):
                """Closures for batch-b prep: x^T, k^T, q^T(spill), v."""
                handles = {}
                handles["xT"] = xT_pool.tile(
                    [128, CT, N], BF16, tag="xT", name=f"xT_{rep}_{b}")
                handles["kT"] = kT_pool.tile(
                    [128, CT, MT * 128], BF16, tag="kT", name=f"kT_{rep}_{b}")
                handles["vst"] = v_pool.tile(
                    [128, NT, C], BF16, tag="vst", name=f"vst_{rep}_{b}")
                xT, kT, vst = handles["xT"], handles["kT"], handles["vst"]
                chunks = []
                xls = [None] * NT
                # streamed qkv weight tiles (bf16, gpsimd casting DMA),
                # prefetched ahead of their consumer chunks.  wkv tiles are
                # [p, ct, 128] stationaries (k then q f-tiles); wv tiles are
                # [p, ct, 512] moving halves for the token-major v matmuls.
                wks = [None] * CT
                wqs = [None] * CT
                wvs = [None] * 2

                def wk_load(ft):
                    wks[ft] = wkv_pool.tile([128, CT, 128], BF16, tag="wkv",
                                            name=f"wk_{rep}_{b}_{ft}")
                    nc.gpsimd.dma_start(
                        out=wks[ft][:],
                        in_=wqkv_d[:, C + ft * 128:C + (ft + 1) * 128
                                   ].rearrange("(co p) f -> p co f", p=128),
                    )

                def wq_load(ft):
                    wqs[ft] = wkv_pool.tile([128, CT, 128], BF16, tag="wkv",
                                            name=f"wq_{rep}_{b}_{ft}")
                    nc.gpsimd.dma_start(
                        out=wqs[ft][:],
                        in_=wqkv_d[:, ft * 128:(ft + 1) * 128
                                   ].rearrange("(co p) f -> p co f", p=128),
                    )

                def wv_load(fh):
                    wvs[fh] = wv_pool.tile([128, CT, 512], BF16, tag="wv",
                                           name=f"wv_{rep}_{b}_{fh}")
                    nc.gpsimd.dma_start(
                        out=wvs[fh][:],
                        in_=wqkv_d[:, 2 * C + fh * 512:2 * C + (fh + 1) * 512
                                   ].rearrange("(co p) f -> p co f", p=128),
                    )

                def lead():
                    for nt in range(2):
                        xls[nt] = xload.tile([128, C], F32, tag="xl", name=f"xl_{rep}_{b}_{nt}")
                        nc.sync.dma_start(
                            out=xls[nt][:],
                            in_=x_d[b, nt * 128:(nt + 1) * 128, :])
                    wk_load(0)
                    wk_load(1)
                chunks.append(lead)

                def xt_chunk(nt):
                    def go():
                        if nt + 2 < NT:
                            xls[nt + 2] = xload.tile([128, C], F32, tag="xl", name=f"xl_{rep}_{b}_{nt+2}")
                            nc.sync.dma_start(
                                out=xls[nt + 2][:],
                                in_=x_d[b, (nt + 2) * 128:(nt + 3) * 128, :])
                        for ch in range(2):
                            ps = ps_prep.tile([128, 512], F32, tag="pp")
                            for i in range(4):
                                ct = 4 * ch + i
                                nc.tensor.transpose(
                                    ps[:, i * 128:(i + 1) * 128],
                                    xls[nt][:, ct * 128:(ct + 1) * 128],
                                    ident[:],
                                )
                            nc.vector.tensor_copy(
                                xT[:, 4 * ch:4 * ch + 4,
                                   nt * 128:(nt + 1) * 128],
                                ps[:].rearrange("p (a c) -> p a c", c=128),
                            )
                    return go
                def prefix():
                    wv_load(0)
                    # zero prefix-tile pad columns 16..128 (scores -> 0,
                    # exp -> 1, but vx mt-0 pad rows are zero so no effect)
                    nc.vector.memset(kT[:, :, P:128], 0.0)
                    pkl = xload.tile([128, C], F32, tag="xl")
                    nc.sync.dma_start(out=pkl[0:P, :], in_=pk_d[b, :, :])
                    for ch in range(2):
                        ps = ps_prep.tile([128, 512], F32, tag="pp")
                        for i in range(4):
                            ct = 4 * ch + i
                            nc.tensor.transpose(
                                ps[:, i * 128:i * 128 + P],
                                pkl[0:P, ct * 128:(ct + 1) * 128],
                                ident[0:P, 0:P],
                            )
                        nc.vector.tensor_copy(
                            kT[:, 4 * ch:4 * ch + 4, 0:P],
                            ps[:].rearrange("p (a c) -> p a c", c=128)[
                                :, :, 0:P],
                        )
                def k_chunk(ft, nh):
                    def go():
                        if nh == 0:
                            if ft < 2:
                                wk_load(ft + 2)        # wk2, wk3
                            elif ft < 4:
                                wq_load(ft - 2)        # wq0, wq1
                            elif ft == 4:
                                wk_load(7)
                        ps = ps_prep.tile([128, 512], F32, tag="pp")
                        for ct in range(CT):
                            nc.tensor.matmul(
                                ps[:],
                                wks[ft][:, ct, :],
                                xT[:, ct, nh * 512:(nh + 1) * 512],
                                start=(ct == 0), stop=(ct == CT - 1),
                            )
                        nc.vector.tensor_copy(
                            kT[:, ft, 128 + nh * 512:128 + (nh + 1) * 512],
                            ps[:],
                        )
                    return go
                # lead-in order: the first four x^T tiles, then k chunks
                # (n-half 0) interleaved with the remaining x^T tiles so the
                # PE has matmul work while x DMAs land.  k f-tiles 4-7 feed
                # attention heads 8+ only, so they are deferred into the
                # attention window.
                for nt in range(4):
                    chunks.append(xt_chunk(nt))
                for i in range(4):
                    chunks.append(k_chunk(i, 0))
                    chunks.append(xt_chunk(4 + i))
                chunks.append(prefix)
                for ft in range(4):
                    chunks.append(k_chunk(ft, 1))

                def q_chunk(ft):
                    def go():
                        if ft == 0:
                            wq_load(2)
                        elif ft < 4:
                            wk_load(ft + 3)            # wk4, wk5, wk6
                            wq_load(ft + 2)            # wq3, wq4, wq5
                        elif ft < 6:
                            wq_load(ft + 2)            # wq6, wq7
                        q_sb = qsb_pool.tile([128, N], BF16, tag="qsb")
                        for nh in range(2):
                            ps = ps_prep.tile([128, 512], F32, tag="pp")
                            for ct in range(CT):
                                nc.tensor.matmul(
                                    ps[:],
                                    wqs[ft][:, ct, :],
                                    xT[:, ct, nh * 512:(nh + 1) * 512],
                                    start=(ct == 0), stop=(ct == CT - 1),
                                )
                            nc.vector.tensor_copy(
                                q_sb[:, nh * 512:(nh + 1) * 512], ps[:])
                        nc.sync.dma_start(out=qsp_d[b, ft], in_=q_sb[:])
                    return go
                chunks.append(q_chunk(0))

                def v_chunk(nt, fh):
                    def go():
                        # token-major v: psum[n, f] = xT-tile^T x w_v
                        ps = ps_prep.tile([128, 512], F32, tag="pp")
                        for ct in range(CT):
                            nc.tensor.matmul(
                                ps[:],
                                xT[:, ct, nt * 128:(nt + 1) * 128],
                                wvs[fh][:, ct, :],
                                start=(ct == 0), stop=(ct == CT - 1),
                            )
                        nc.vector.tensor_copy(
                            vst[:, nt, fh * 512:(fh + 1) * 512], ps[:])
                    return go
                for nt in range(NT):
                    chunks.append(v_chunk(nt, 0))

                # deferred chunks: only needed by later attention heads of
                # this batch, so they are injected into the attention window
                # itself (q(ft)/k(ft) feed head pair ft; v fh=1 feeds heads
                # 8-15); ordered so every chunk lands before its deadline at
                # the standard cadence
                # q spills go EARLY (their qp reloads race through DRAM
                # scratch; keep tens of microseconds of margin), k/v chunks
                # follow, all meeting their head deadlines at cadence 4
                deferred = [q_chunk(1), q_chunk(2), lambda: wv_load(1),
                            q_chunk(3), v_chunk(0, 1), q_chunk(4),
                            v_chunk(1, 1), k_chunk(4, 0), v_chunk(2, 1),
                            k_chunk(4, 1), v_chunk(3, 1), q_chunk(5),
                            v_chunk(4, 1), v_chunk(5, 1), k_chunk(5, 0),
                            v_chunk(6, 1), k_chunk(5, 1), v_chunk(7, 1),
                            q_chunk(6), k_chunk(6, 0), k_chunk(6, 1),
                            q_chunk(7), k_chunk(7, 0), k_chunk(7, 1)]

                return chunks, deferred, handles

            def attn_steps(rep, b, handles):
                """Generator: batch-b attention, one PE-step per yield."""
                kT, vst = handles["kT"], handles["vst"]
                oT = oT_pool.tile(
                    [128, CT, N], BF16, tag="oT", name=f"oT_{rep}_{b}")
                handles["oT"] = oT
                qps = [None] * HPAIRS

                def qp_load(pair):
                    qps[pair] = qp_pool.tile([128, N], BF16, tag="qp",
                                             name=f"qp_{rep}_{b}_{pair}")
                    nc.sync.dma_start(out=qps[pair][:], in_=qsp_d[b, pair])

                qp_load(0)
                for h in range(H):
                    hp, base = h // 2, 64 * (h % 2)
                    # prefetch the next pair's q at the ODD head: late
                    # enough that the deferred q-spill chunk feeding it has
                    # already been injected (same-queue RAW would deadlock),
                    # early enough to hide the DMA
                    if h % 2 == 1 and hp + 1 < HPAIRS:
                        qp_load(hp + 1)
                    qp = qps[hp]
                    # build vx for this head: [m, 0:64]=v_h, [m, 64:128]=ones
                    # (denominator trick); mt 0 rows P:128 are zero pads
                    vx = vx_pool.tile([128, MT, 128], BF16, tag="vx",
                                      name=f"vx_{rep}_{b}_{h}")
                    nc.vector.memset(vx[:, 0, :], 0.0)
                    nc.vector.memset(vx[:, 1:MT, 64:128], 1.0)
                    nc.vector.memset(vx[0:P, 0, 64:128], 1.0)
                    nc.gpsimd.dma_start(
                        out=vx[0:P, 0, 0:64],
                        in_=pv_d[b, :, h * D:(h + 1) * D],
                    )
                    nc.vector.tensor_copy(
                        vx[:, 1:MT, 0:64], vst[:, :, h * D:(h + 1) * D])

                    ps_av = ps_av_pool.tile([128, N], F32, tag="av",
                                            name=f"av_{rep}_{b}_{h}")
                    eTs = [None] * MT

                    def sc(mt):
                        ps = ps_sc.tile([128, N], F32, tag="sc")
                        for j in range(0, N, 512):
                            nc.tensor.matmul(
                                ps[:, j:j + 512],
                                kT[base:base + D, hp,
                                   mt * 128:(mt + 1) * 128],
                                qp[base:base + D, j:j + 512],
                                start=True, stop=True,
                            )
                        eTs[mt] = e_pool.tile([128, N], BF16, tag="eT", name=f"eT_{rep}_{b}_{h}_{mt}")
                        nc.scalar.activation(eTs[mt][:], ps[:], AF.Exp,
                                             scale=SCALE)

                    def av(mt):
                        for j in range(0, N, 512):
                            nc.tensor.matmul(
                                ps_av[:, j:j + 512],
                                vx[:, mt, :],
                                eTs[mt][:, j:j + 512],
                                start=(mt == 0), stop=(mt == MT - 1),
                            )
                        eTs[mt] = None

                    sc(0)
                    yield
                    for mt in range(1, MT):
                        sc(mt)
                        av(mt - 1)
                        yield
                    av(MT - 1)
                    # one copy frees the PSUM accumulator ASAP (next head's
                    # attn@v only waits on this); rows 64:128 hold the
                    # softmax denominator (replicated).  1/den is computed
                    # as exp(-ln(den)) on ACT -- both in one activation
                    # table set (see pinning patch above), no reloads --
                    # instead of the very slow DVE reciprocal.
                    stg = stg_pool.tile([128, N], F32, tag="stg")
                    nc.vector.tensor_copy(stg[:], ps_av[:])
                    rb = rb_pool.tile([64, N], F32, tag="rb")
                    nc.scalar.activation(rb[:], stg[64:128, :], AF.Ln)
                    nc.scalar.activation(rb[:], rb[:], AF.Exp, scale=-1.0)
                    nc.vector.tensor_mul(
                        oT[base:base + 64, hp, :], stg[0:64, :], rb[:])
                    yield

            def proj_chunks(rep, b, handles):
                oT = handles["oT"]
                chunks = []
                wps = [None] * CT

                def wp_load(cft):
                    wps[cft] = wp_pool.tile([128, CT, 128], BF16, tag="wp",
                                            name=f"wp_{rep}_{b}_{cft}")
                    nc.gpsimd.dma_start(
                        out=wps[cft][:],
                        in_=wproj_d[:, cft * 128:(cft + 1) * 128
                                    ].rearrange("(co p) f -> p co f", p=128),
                    )

                wp_load(0)

                def p_chunk(cft, nh):
                    def go():
                        # prefetch next tile's weights so the matmuls there
                        # never wait on their own DMA
                        if nh == 1 and cft + 1 < CT:
                            wp_load(cft + 1)
                        ps = ps_prep.tile([128, 512], F32, tag="pp")
                        for ct in range(CT):
                            nc.tensor.matmul(
                                ps[:],
                                wps[cft][:, ct, :],
                                oT[:, ct, nh * 512:(nh + 1) * 512],
                                start=(ct == 0), stop=(ct == CT - 1),
                            )
                        o_sb = osb_pool.tile([128, 512], F32, tag="osb")
                        nc.vector.tensor_scalar_add(
                            o_sb[:], ps[:], bias_col[:, cft:cft + 1])
                        nc.sync.dma_start(
                            out=outT_d[b, cft * 128:(cft + 1) * 128,
                                       nh * 512:(nh + 1) * 512],
                            in_=o_sb[:],
                        )
                    return go
                for cft in range(CT):
                    for nh in range(2):
                        chunks.append(p_chunk(cft, nh))
                return chunks

            def run_interleaved(steps, fills):
                """Emit attention steps; `fills` is a list of
                (chunk_list, cadence[, start]) entries -- from step `start`
                on, every `cadence` steps one chunk from that list is
                injected as PE filler for the ACT-bound stretches."""
                i = 0
                for _ in steps:
                    i += 1
                    for entry in fills:
                        fl, cad = entry[0], entry[1]
                        start = entry[2] if len(entry) > 2 else 0
                        if fl and i >= start and (i - start) % cad == 0:
                            fl.pop(0)()
                for entry in fills:
                    for c in entry[0]:
                        c()

            for rep in range(repeat):
                # window A: batch-0 early prep (x^T, k^T, q0, v half 0)
                pre0, def0, h0 = prep_chunks(rep, 0)
                for c in pre0:
                    c()
                # window B: batch-0 attention + (batch-0 late prep,
                # batch-1 early prep) interleaved
                pre1, def1, h1 = prep_chunks(rep, 1)
                run_interleaved(attn_steps(rep, 0, h0),
                                [[def0, 4], [pre1, 5]])
                # window C: batch-1 attention + (batch-1 late prep,
                # batch-0 proj halves) interleaved
                run_interleaved(attn_steps(rep, 1, h1),
                                [[def1, 4],
                                 [proj_chunks(rep, 0, h0), 8, 40]])
                # window D: batch-1 proj, solo
                for c in proj_chunks(rep, 1, h1):
                    c()

    return nc


_NC_CACHE = {}


def _get_nc(repeat: int = 1) -> bass.Bass:
    key = f"nc{repeat}"
    if key not in _NC_CACHE:
        _NC_CACHE[key] = build_nc(repeat)
    return _NC_CACHE[key]


def _make_runner(nc):
    """Compile the SPMD kernel ONCE into a reusable callable.

    Mirrors bass2jax.run_bass_via_pjrt's multi-core branch, but without
    output-buffer donation so the compiled function + device-resident
    inputs can be invoked repeatedly (for wall-clock benchmarking and to
    avoid recompiles on every kernel() call).
    """
    import jax
    from jax.experimental.shard_map import shard_map
    from jax.sharding import Mesh, PartitionSpec
    from concourse import bass2jax
    from concourse.bass2jax import _bass_exec_p, partition_id_tensor

    bass2jax.install_neuronx_cc_hook()

    partition_name = (
        nc.partition_id_tensor.name if nc.partition_id_tensor else None
    )
    in_names, out_names, out_avals, zero_outs = [], [], [], []
    for alloc in nc.m.functions[0].allocations:
        if not isinstance(alloc, mybir.MemoryLocationSet):
            continue
        name = alloc.memorylocations[0].name
        if alloc.kind == "ExternalInput":
            if name != partition_name:
                in_names.append(name)
        elif alloc.kind == "ExternalOutput":
            shape = tuple(alloc.tensor_shape)
            dtype = mybir.dt.np(alloc.dtype)
            out_names.append(name)
            out_avals.append(jax.core.ShapedArray(shape, dtype))
            zero_outs.append(np.zeros(shape, dtype))
    n_params = len(in_names)
    all_in_names = list(in_names) + list(out_names)
    if partition_name is not None:
        all_in_names.append(partition_name)

    def _body(*args):
        operands = list(args)
        if partition_name is not None:
            operands.append(partition_id_tensor())
        outs = _bass_exec_p.bind(
            *operands,
            out_avals=tuple(out_avals),
            in_names=tuple(all_in_names),
            out_names=tuple(out_names),
            lowering_input_output_aliases=(),
            sim_require_finite=True,
            sim_require_nnan=True,
            nc=nc,
        )
        return tuple(outs)

    devices = jax.devices()[:N_CORES]
    mesh = Mesh(np.asarray(devices), ("core",))
    n_outs = len(out_avals)
    in_specs = (PartitionSpec("core"),) * (n_params + n_outs)
    out_specs = (PartitionSpec("core"),) * n_outs
    sharded = jax.jit(
        shard_map(_body, mesh=mesh, in_specs=in_specs,
                  out_specs=out_specs, check_rep=False),
        keep_unused=True,
    )

    concat_zeros = [
        np.zeros((N_CORES * z.shape[0], *z.shape[1:]), z.dtype)
        for z in zero_outs
    ]

    state = {"dev_zeros": None}

    def runner(in_maps):
        per_core = [
            [np.asarray(m[name]) for name in in_names] for m in in_maps
        ]
        concat_in = [
            np.concatenate([per_core[c][i] for c in range(N_CORES)], axis=0)
            for i in range(n_params)
        ]
        if state["dev_zeros"] is None:
            state["dev_zeros"] = [jax.device_put(z) for z in concat_zeros]
        out_arrs = sharded(*concat_in, *state["dev_zeros"])
        return [
            {
                name: np.asarray(out_arrs[i]).reshape(
                    N_CORES, *out_avals[i].shape
                )[c]
                for i, name in enumerate(out_names)
            }
            for c in range(N_CORES)
        ]

    def runner_dev(dev_args):
        """dev_args: device-resident concat inputs; returns device outputs."""
        return sharded(*dev_args, *state["dev_zeros"])

    def make_dev_args(in_maps):
        per_core = [
            [np.asarray(m[name]) for name in in_names] for m in in_maps
        ]
        concat_in = [
            np.concatenate([per_core[c][i] for c in range(N_CORES)], axis=0)
            for i in range(n_params)
        ]
        if state["dev_zeros"] is None:
            state["dev_zeros"] = [jax.device_put(z) for z in concat_zeros]
        return [jax.device_put(a) for a in concat_in]

    return runner, runner_dev, make_dev_args


def _get_runner(repeat: int = 1):
    key = f"runner{repeat}"
    if key not in _NC_CACHE:
        _NC_CACHE[key] = _make_runner(_get_nc(repeat))
    return _NC_CACHE[key]


def _make_in_maps(x, pk, pv, w_qkv, w_proj, b_proj):
    x = np.ascontiguousarray(np.asarray(x, dtype=np.float32))
    pk = np.ascontiguousarray(np.asarray(pk, dtype=np.float32))
    pv = np.ascontiguousarray(np.asarray(pv, dtype=np.float32))
    w_qkv = np.ascontiguousarray(np.asarray(w_qkv, dtype=np.float32))
    w_proj = np.ascontiguousarray(np.asarray(w_proj, dtype=np.float32))
    b_proj = np.ascontiguousarray(np.asarray(b_proj, dtype=np.float32))
    in_maps = []
    for c in range(N_CORES):
        sl = slice(c * B_PC, (c + 1) * B_PC)
        in_maps.append({
            "x": x[sl], "pk": pk[sl], "pv": pv[sl],
            "w_qkv": w_qkv, "w_proj": w_proj, "b_proj": b_proj,
        })
    return in_maps


def run(x, pk, pv, w_qkv, w_proj, b_proj, trace=False, **trace_kwargs):
    """Run the SPMD kernel; returns (output [B,N,C], per-core results).

    With trace=True, runs through run_bass_kernel_spmd so the NTFF
    profile hook captures HW exec time; returns (out, BassKernelResults).
    """
    in_maps = _make_in_maps(x, pk, pv, w_qkv, w_proj, b_proj)
    if trace:
        res = run_bass_kernel_spmd(
            _get_nc(), in_maps, core_ids=list(range(N_CORES)),
            trace=True, **trace_kwargs,
        )
        results = res.results
    else:
        runner, _, _ = _get_runner()
        results = runner(in_maps)
        res = results
    out = np.empty((B, N, C), dtype=np.float32)
    for c in range(N_CORES):
        outT = results[c]["outT"]              # [B_PC, C, N]
        out[c * B_PC:(c + 1) * B_PC] = outT.transpose(0, 2, 1)
    return out, res


def kernel(x, pk, pv, w_qkv, w_proj, b_proj) -> np.ndarray:
    out, _ = run(x, pk, pv, w_qkv, w_proj, b_proj)
    return out


def benchmark(x, pk, pv, w_qkv, w_proj, b_proj, iters=20, warmup=3, repeat=1):
    """Median wall-clock per executed call with device-resident inputs."""
    import time
    import jax
    _, runner_dev, make_dev_args = _get_runner(repeat)
    in_maps = _make_in_maps(x, pk, pv, w_qkv, w_proj, b_proj)
    dev_args = make_dev_args(in_maps)
    for _ in range(warmup):
        outs = runner_dev(dev_args)
        jax.block_until_ready(outs)
    ts = []
    for _ in range(iters):
        t0 = time.perf_counter()
        outs = runner_dev(dev_args)
        jax.block_until_ready(outs)
        ts.append(time.perf_counter() - t0)
    ts.sort()
    return {
        "median_s": ts[len(ts) // 2],
        "min_s": ts[0],
        "all_s": ts,
    }

